# revision 79
# baseline (speedup 1.0000x reference)
"""Transformer block (pre-LN MHA + FFN) Trainium2 Bass kernel, fp8 edition.

Data-parallel over 8 cores: core c handles batch b=c//2, sequence half c%2.
Each core computes LN1 + K/V over the batch's FULL 2048 rows, Q/attention/
LN2/FFN over its own 1024 rows (rolled to columns 0:1024 host-side).

All heavy matmuls use fp8e4m3 DoubleRow perf mode (2 stacked 128-deep
contractions per instruction at 0.5 cycles/row):
 - QKV projections / FFN contract pairs of d-tiles of the fp8 activations.
 - Attention scores (64-deep per head) use a zero second stack on the Q
   side (step-sliced view onto a zeroed 9th tile) for 2x.
 - attn@V stacks pairs of k-row tiles of exp(scores) in fp8.
FFN weights are split hi+lo fp8 host-side (error compensation); the relu
activations optionally get the same split (BK_FFN2MM=3).  exp() is scaled
by 1/16 (bias -ln16) so unnormalized attn fits fp8.

LN rstd = exp(-0.5*ln(var+eps)) keeps the ACT engine on a single table
(natural_log_exp_and_others) so the softmax exp stream never reloads.

Emission interleaves FFN/LN2/normalize work for query-chunk pair p under
the ACT-bound attention windows of later chunks (background task queue).

Self-contained: hardcodes shapes B=4, S=2048, D=1024, H=16, FF=4096.
"""

import os

import numpy as np
import ml_dtypes

import concourse.bass as bass
import concourse.bacc as bacc
import concourse.tile as tile
from concourse import mybir

F32 = mybir.dt.float32
BF16 = mybir.dt.bfloat16
F8 = mybir.dt.float8e4
AF = mybir.ActivationFunctionType
OP = mybir.AluOpType
DRM = mybir.MatmulPerfMode.DoubleRow

B, S, D, H, FF = 4, 2048, 1024, 16, 4096
HD = D // H          # 64
P = 128
DT = D // P          # 8  d-tiles
FT = FF // P         # 32 ff-tiles
KT = S // P          # 16 k-row tiles
SQ = S // 2          # 1024 own q columns per core
AQ = 256             # attention q-chunk
NQC = SQ // AQ       # 4
EPS = 1e-5
EXPB = -5.545177444479562   # -ln(256): scale exp so fp8 numerator is safe
NCORES = 8

FFN1MM = int(os.environ.get("BK_FFN1MM", "3"))
FFN2MM = int(os.environ.get("BK_FFN2MM", "3"))
# Host-side weight scales keep fp8e4m3 (max 240, min normal 2^-6) in its
# normal range; descaled at evictions / the denominator stage.
WS_QKV = 32.0        # Wq/Wk scale (q,k descaled at evict)
WS_V = 2.0           # Wv scale (rides into attn numerator; denom stage x2)
WS_FFN1 = 32.0       # W1 scale (relu stage keeps 32x)
WS_FFN2 = 64.0       # W2 scale (final evict divides 32*64)

_CACHE = {}


def _build_nc():
    nc = bacc.Bacc("TRN2", target_bir_lowering=False, debug=False,
                   num_devices=NCORES)

    xbf = nc.dram_tensor("xbf", [P, DT, S], BF16, kind="ExternalInput")
    xf8 = nc.dram_tensor("xf8", [P, DT, S], F8, kind="ExternalInput")
    xsq8 = nc.dram_tensor("xsq8", [P, DT, S], F8, kind="ExternalInput")
    xh = nc.dram_tensor("xh", [P, DT, SQ], BF16, kind="ExternalInput")
    wqk = nc.dram_tensor("wqk", [P, 2, DT, D], F8, kind="ExternalInput")
    wv = nc.dram_tensor("wv", [P, DT, D], F8, kind="ExternalInput")
    w1x = nc.dram_tensor("w1x", [P, 2, DT, FF], F8, kind="ExternalInput")
    w2x = nc.dram_tensor("w2x", [P, 2, FT, D], F8, kind="ExternalInput")
    g2d = nc.dram_tensor("g2d", [P, DT, P], BF16, kind="ExternalInput")
    bq = nc.dram_tensor("bq", [P, DT], F32, kind="ExternalInput")
    bk = nc.dram_tensor("bk", [P, DT], F32, kind="ExternalInput")
    bvb = nc.dram_tensor("bvb", [P, D], BF16, kind="ExternalInput")
    b1 = nc.dram_tensor("b1", [P, FT], F32, kind="ExternalInput")
    b2 = nc.dram_tensor("b2", [P, DT], F32, kind="ExternalInput")
    emat = nc.dram_tensor("emat", [16, DT, P], BF16, kind="ExternalInput")
    OUT = nc.dram_tensor("OUT", [P, DT, SQ], F32, kind="ExternalOutput")

    repeat = int(os.environ.get("BASS_KERNEL_REPEAT", "1"))
    with tile.TileContext(nc) as tc:
        for _ in range(repeat):
            _emit(nc, tc, xbf, xf8, xsq8, xh, wqk, wv, w1x, w2x, g2d,
                  bq, bk, bvb, b1, b2, emat, OUT)
    nc.compile()
    return nc


def _emit(nc, tc, xbf_d, xf8_d, xsq8_d, xh_d, wqk_d, wv_d, w1x_d, w2x_d,
          g2d_d, bq_d, bk_d, bvb_d, b1_d, b2_d, emat_d, OUT_d):
    pools = {}
    pobj = {}

    def open_pool(name, bufs, space="SBUF"):
        cm = tc.tile_pool(name=name, bufs=bufs, space=space)
        pools[name] = cm
        pobj[name] = cm.__enter__()
        return pobj[name]

    def close_pool(name):
        pools.pop(name).__exit__(None, None, None)

    def dr(ps, lhsT, rhs, start, stop):
        nc.tensor.matmul(ps, lhsT, rhs, start=start, stop=stop,
                         perf_mode=DRM, skip_group_check=True)

    p_const = open_pool("consts", 1)
    p_ps = open_pool("psg", 2, space="PSUM")
    p_sc = open_pool("scps", 2, space="PSUM")
    p_aps = open_pool("apsps", 2, space="PSUM")

    # ---- constants ----
    ones8 = p_const.tile([P, 2, P], F8, tag="ones8")
    nc.vector.memset(ones8[:], 1.0)
    ones_bf = p_const.tile([P, P], BF16, tag="ones")
    nc.vector.memset(ones_bf[:], 1.0)
    eps_t = p_const.tile([P, 1], F32, tag="eps")
    nc.vector.memset(eps_t[:], EPS)
    expb_t = p_const.tile([P, 1], F32, tag="expb")
    nc.vector.memset(expb_t[:], EXPB)
    sb_bq = p_const.tile([P, DT], F32, tag="bq")
    nc.sync.dma_start(sb_bq[:], bq_d[:, :])
    sb_bk = p_const.tile([P, DT], F32, tag="bk")
    nc.sync.dma_start(sb_bk[:], bk_d[:, :])
    sb_bvb = p_const.tile([P, D], BF16, tag="bvb")
    nc.sync.dma_start(sb_bvb[:], bvb_d[:, :])
    sb_b1 = p_const.tile([P, FT], F32, tag="b1")
    nc.sync.dma_start(sb_b1[:], b1_d[:, :])
    sb_b2 = p_const.tile([P, DT], F32, tag="b2")
    nc.sync.dma_start(sb_b2[:], b2_d[:, :])
    sb_emat = p_const.tile([16, DT, P], BF16, tag="emat")
    nc.sync.dma_start(sb_emat[:], emat_d[:, :, :])
    sb_g2d = p_const.tile([P, DT, P], BF16, tag="g2d")
    nc.sync.dma_start(sb_g2d[:], g2d_d[:, :, :])

    # ---- persistent activations ----
    p_fT = open_pool("fTp", 1)
    fT_bf = p_fT.tile([P, DT, SQ], BF16, tag="fTbf")
    fT8 = p_fT.tile([P, DT, SQ], F8, tag="fT8")
    fT8lo = (p_fT.tile([P, DT, SQ], F8, tag="fT8lo", name="fT8lo")
             if FFN1MM == 3 else None)

    p_qkv = open_pool("qkvout", 1)
    qT8 = p_qkv.tile([P, DT + 1, SQ], F8, tag="qT8")
    nc.vector.memset(qT8[:, DT, :], 0.0)
    kT8 = p_qkv.tile([P, DT + 1, S], F8, tag="kT8")
    nc.vector.memset(kT8[:, DT, :], 0.0)
    vaug = p_qkv.tile([P, KT, H, HD + 1], F8, tag="vaug")
    nc.vector.memset(vaug[:, :, :, HD:HD + 1], 1.0)

    p_hT = open_pool("hTp", 1)
    hT8 = p_hT.tile([P, DT, S], F8, tag="hT8")

    p_w = open_pool("wslab", 1)
    wv_s = p_w.tile([P, DT, D], F8, tag="wv_s")
    nc.sync.dma_start(wv_s[:], wv_d[:, :, :])

    p_x = open_pool("xp", 1)
    xbf = p_x.tile([P, DT, S], BF16, tag="xbf")
    xf8 = p_x.tile([P, DT, S], F8, tag="xf8")
    xsq8 = p_x.tile([P, DT, S], F8, tag="xsq8")
    p_lt = open_pool("ln1tmp", 1)

    def ln_stats_smalls(ps1, ps2, tmp_pool, sfx, w):
        """psum sums -> (rstd bf16, nsb bf16) tiles of width w."""
        mu = tmp_pool.tile([P, w], F32, tag="mu" + sfx)
        nc.scalar.activation(mu[:], ps1, AF.Copy, bias=0.0, scale=1.0 / D)
        msq = tmp_pool.tile([P, w], F32, tag="msq" + sfx)
        nc.scalar.activation(msq[:], ps2, AF.Copy, bias=0.0, scale=1.0 / D)
        var = tmp_pool.tile([P, w], F32, tag="var" + sfx)
        nc.vector.tensor_mul(var[:], mu[:], mu[:])
        nc.vector.tensor_sub(var[:], msq[:], var[:])
        lnv = tmp_pool.tile([P, w], F32, tag="lnv" + sfx)
        nc.scalar.activation(lnv[:], var[:], AF.Ln, bias=eps_t[:], scale=1.0)
        rstd = tmp_pool.tile([P, w], BF16, tag="rstd" + sfx)
        with nc.allow_low_precision(reason="rstd bf16 feeds bf16 multiplies"):
            nc.scalar.activation(rstd[:], lnv[:], AF.Exp, bias=0.0, scale=-0.5)
        negmu = tmp_pool.tile([P, w], F32, tag="negmu" + sfx)
        nc.scalar.activation(negmu[:], mu[:], AF.Copy, bias=0.0, scale=-1.0)
        nsb = tmp_pool.tile([P, w], BF16, tag="nsb" + sfx)
        nc.vector.tensor_mul(nsb[:], negmu[:], rstd[:])
        return rstd, nsb

    # =========================================================
    # Phase A: LN1 (fp8 DR stats via host x/x^2) + V projection
    # =========================================================
    wqk_s = p_w.tile([P, 2, DT, D], F8, tag="wqk_s")
    nc.sync.dma_start(wqk_s[:], wqk_d[:, :, :, :])

    for sc in range(4):
        ssl = bass.ts(sc, 512)
        nc.sync.dma_start(xbf[:, :, ssl], xbf_d[:, :, ssl])
        nc.sync.dma_start(xf8[:, :, ssl], xf8_d[:, :, ssl])
        nc.sync.dma_start(xsq8[:, :, ssl], xsq8_d[:, :, ssl])

        ps12 = p_sc.tile([P, 2, 512], F32, tag="scps")
        for i in range(4):
            dr(ps12[:, 0, :], ones8[:, :, :], xf8[:, 2 * i:2 * i + 2, ssl],
               start=(i == 0), stop=(i == 3))
        for i in range(4):
            dr(ps12[:, 1, :], ones8[:, :, :], xsq8[:, 2 * i:2 * i + 2, ssl],
               start=(i == 0), stop=(i == 3))
        rstd, nsb = ln_stats_smalls(ps12[:, 0, :], ps12[:, 1, :], p_lt,
                                    "a", 512)
        tmpb = p_lt.tile([P, DT, 512], BF16, tag="tmpb")
        nc.vector.tensor_tensor(
            tmpb[:], xbf[:, :, ssl],
            rstd[:, None, :].to_broadcast((P, DT, 512)), OP.mult)
        with nc.allow_low_precision(reason="hT fp8 matches matmul dtype"):
            nc.gpsimd.tensor_tensor(
                hT8[:, :, ssl], tmpb[:],
                nsb[:, None, :].to_broadcast((P, DT, 512)), OP.add)

        # V (Pool evict), K (ACT evict), Q (DVE evict) for this chunk
        for kt in range(4 * sc, 4 * sc + 4):
            for g in range(2):
                pv = p_ps.tile([P, 512], F32, tag="psg")
                for i in range(4):
                    dr(pv[:], hT8[:, 2 * i:2 * i + 2, bass.ts(kt, P)],
                       wv_s[:, 2 * i:2 * i + 2, bass.ts(g, 512)],
                       start=(i == 0), stop=(i == 3))
                with nc.allow_low_precision(reason="v fp8 for fp8 attn"):
                    nc.vector.tensor_tensor(
                        vaug[:, kt, 8 * g:8 * g + 8, 0:HD],
                        pv[:].rearrange("p (h d) -> p h d", d=HD),
                        sb_bvb[:, bass.ts(g, 512)].rearrange(
                            "p (h d) -> p h d", d=HD),
                        OP.add)
        for t in range(DT):
            pk = p_sc.tile([P, 512], F32, tag="scps")
            for i in range(4):
                dr(pk[:], wqk_s[:, 1, 2 * i:2 * i + 2, bass.ts(t, P)],
                   hT8[:, 2 * i:2 * i + 2, ssl],
                   start=(i == 0), stop=(i == 3))
            with nc.allow_low_precision(reason="k fp8 for fp8 attn"):
                nc.scalar.activation(kT8[:, t, ssl], pk[:],
                                     AF.Identity, bias=sb_bk[:, t:t + 1],
                                     scale=1.0 / WS_QKV)
            if sc < 2:
                pq = p_ps.tile([P, 512], F32, tag="psg")
                for i in range(4):
                    dr(pq[:], wqk_s[:, 0, 2 * i:2 * i + 2, bass.ts(t, P)],
                       hT8[:, 2 * i:2 * i + 2, ssl],
                       start=(i == 0), stop=(i == 3))
                with nc.allow_low_precision(reason="q fp8 for fp8 attn"):
                    if t % 2 == 0:
                        nc.scalar.activation(qT8[:, t, ssl], pq[:],
                                             AF.Identity,
                                             bias=sb_bq[:, t:t + 1],
                                             scale=1.0 / WS_QKV)
                    else:
                        nc.vector.tensor_scalar(qT8[:, t, ssl], pq[:],
                                                1.0 / WS_QKV,
                                                sb_bq[:, t:t + 1],
                                                OP.mult, OP.add)

    close_pool("ln1tmp")
    close_pool("xp")
    close_pool("wslab")
    close_pool("hTp")

    # =========================================================
    # Phase C/D/E: attention chunks with interleaved background
    # normalize/LN2/FFN work for earlier chunks.
    # =========================================================
    p_att = open_pool("attn", 1)
    attn8 = p_att.tile([P, DT, SQ], F8, tag="attn8")
    p_ex = open_pool("expT", 2)
    p_st = open_pool("stage", 2)

    p_xh = open_pool("xhp", 1)
    xh = p_xh.tile([P, DT, SQ], BF16, tag="xh")
    for dt_ in range(DT):
        nc.sync.dma_start(xh[:, dt_, :], xh_d[:, dt_, :])
    rpad = p_xh.tile([16, SQ], BF16, tag="rpad")
    nc.vector.memset(rpad[:], 0.0)   # emat matmul reads all rows; keep finite

    p_yb = open_pool("ybp", 1)
    ybf = p_yb.tile([P, DT, SQ], BF16, tag="ybf")
    p_l2 = open_pool("ln2tmp", 1)

    p_fw = open_pool("ffnw", 2)
    p_rl = open_pool("relu", 1)
    relu8 = p_rl.tile([P, FT, 512], F8, tag="relu8")
    relu8lo = (p_rl.tile([P, FT, 512], F8, tag="relu8lo", name="relu8lo")
               if FFN2MM == 3 else None)
    p_rbf = open_pool("relubf", 2)
    p_fo = open_pool("fout", 2)

    bg = []
    pools_cur = {"ps": p_ps, "sc": p_sc}

    def drain(n):
        for _ in range(min(n, len(bg))):
            bg.pop(0)()

    def attn_scores(t, i, qc):
        """Score burst + exp for head (t, i); returns the per-head ex tile."""
        pb = 64 * i
        qsl = bass.ts(qc, AQ)
        exh = p_ex.tile([P, 16, AQ], F8, tag="expT")
        for g in range(4):
            sc_ps = p_sc.tile([P, 4, AQ], F32, tag="scps")
            for kk in range(4):
                kt = 4 * g + kk
                nc.tensor.matmul(
                    sc_ps[:, kk, :],
                    kT8[pb:pb + 64, t:t + 2, bass.ts(kt, P)],
                    qT8[pb:pb + 64, t:DT + 1:DT - t, qsl],
                    start=(kk % 2 == 0), stop=(kk % 2 == 1),
                    perf_mode=DRM, skip_group_check=True)
            nc.scalar.activation(exh[:, 4 * g:4 * g + 4, :], sc_ps[:], AF.Exp,
                                 bias=expb_t[:], scale=0.125)
        return exh

    def attn_v(t, i, qc, exh):
        """attn@V + evict for head (t, i) using its exp tile."""
        h = 2 * t + i
        pb = 64 * i
        qsl = bass.ts(qc, AQ)
        aps = p_aps.tile([HD + 1, AQ], F32, tag="aps")
        for m in range(8):
            dr(aps[:, :], vaug[:, 2 * m:2 * m + 2, h, :],
               exh[:, 2 * m:2 * m + 2, :],
               start=(m == 0), stop=(m == 7))
        st = p_st.tile([HD, AQ], F8, tag="stage")
        with nc.allow_low_precision(reason="unnormalized attn fp8 (scaled)"):
            nc.vector.tensor_copy(st[:], aps[0:HD, :])
        std = p_st.tile([1, AQ], BF16, tag="staged")
        # denom * WS_V so rpad = 1/(WS_V * den) matches the v scale
        nc.vector.tensor_scalar(std[:], aps[HD:HD + 1, :], WS_V, None,
                                OP.mult)
        rp1 = p_st.tile([1, AQ], BF16, tag="stager")
        with nc.allow_low_precision(reason="softmax denom recip bf16"):
            nc.vector.reciprocal(rp1[:], std[:])
        nc.sync.dma_start(attn8[pb:pb + 64, t, qsl], st[:, :])
        nc.sync.dma_start(rpad[h:h + 1, qsl], rp1[:, :])
        if i == 1:
            bg.append(mk_norm_task(qc, t))
            if t == DT - 1:
                bg.extend(mk_ln2_tasks(qc))
                if qc % 2 == 1:
                    pr = qc // 2
                    for ft in range(FT):
                        bg.append(mk_ffn1_task(pr, ft))
                    for mt in range(DT):
                        bg.extend(mk_ffn2_tasks(pr, mt))

    def mk_norm_task(qc, t):
        """Normalize + residual for d-tile t of chunk qc (heads 2t, 2t+1)."""
        def task():
            qsl = bass.ts(qc, AQ)
            rb = pools_cur["ps"].tile([P, AQ], F32, tag="psg", name="rb")
            nc.tensor.matmul(rb[:], sb_emat[:, t, :], rpad[:, qsl],
                             start=True, stop=True)
            t1 = p_l2.tile([P, AQ], F32, tag="t1")
            nc.vector.tensor_mul(t1[:], attn8[:, t, qsl], rb[:])
            nc.vector.tensor_add(ybf[:, t, qsl], t1[:], xh[:, t, qsl])
        return task

    def mk_ln2_tasks(qc):
        qsl = bass.ts(qc, AQ)
        st_ = {}

        def part_a():
            ysq = p_l2.tile([P, DT, AQ], BF16, tag="scr8a")
            nc.vector.tensor_mul(ysq[:], ybf[:, :, qsl], ybf[:, :, qsl])
            pool = pools_cur["sc"]
            ps12 = pool.tile([P, 2, AQ], F32,
                             tag=("scps" if pool is p_sc else "psg"),
                             name="ln2ps")
            for dt_ in range(DT):
                nc.tensor.matmul(ps12[:, 0, :], ones_bf[:], ybf[:, dt_, qsl],
                                 start=(dt_ == 0), stop=(dt_ == DT - 1))
            for dt_ in range(DT):
                nc.tensor.matmul(ps12[:, 1, :], ones_bf[:], ysq[:, dt_, :],
                                 start=(dt_ == 0), stop=(dt_ == DT - 1))
            st_["ps"] = ps12

        def part_b():
            ps12 = st_["ps"]
            rstd, nsb = ln_stats_smalls(ps12[:, 0, :], ps12[:, 1, :],
                                        p_l2, "b", AQ)
            tmpb = p_l2.tile([P, DT, AQ], BF16, tag="scr8a")
            nc.vector.tensor_tensor(
                tmpb[:], ybf[:, :, qsl],
                rstd[:, None, :].to_broadcast((P, DT, AQ)), OP.mult)
            nc.vector.tensor_tensor(
                fT_bf[:, :, qsl], tmpb[:],
                nsb[:, None, :].to_broadcast((P, DT, AQ)), OP.add)
            with nc.allow_low_precision(reason="f fp8 for fp8 FFN"):
                nc.gpsimd.tensor_tensor(
                    fT8[:, :, qsl], tmpb[:],
                    nsb[:, None, :].to_broadcast((P, DT, AQ)), OP.add)
                if fT8lo is not None:
                    nc.gpsimd.tensor_tensor(fT8lo[:, :, qsl],
                                            fT_bf[:, :, qsl],
                                            fT8[:, :, qsl], OP.subtract)
        return [part_a, part_b]

    def mk_ffn1_task(pr, ft):
        def task():
            psl = bass.ts(pr, 512)
            w1_s = p_fw.tile([P, 2, DT, P], F8, tag="w1s")
            nc.sync.dma_start(w1_s[:], w1x_d[:, :, :, bass.ts(ft, P)])
            pf = pools_cur["ps"].tile([P, 512], F32, tag="psg", name="pf")
            mms = [(0, fT8), (1, fT8)]
            if FFN1MM == 3:
                mms.append((0, fT8lo))
            nmm = 0
            tot = 4 * len(mms)
            for hl, rhs in mms:
                for i in range(4):
                    dr(pf[:], w1_s[:, hl, 2 * i:2 * i + 2, :],
                       rhs[:, 2 * i:2 * i + 2, psl],
                       start=(nmm == 0), stop=(nmm == tot - 1))
                    nmm += 1
            rbf = p_rbf.tile([P, 512], BF16, tag="rbf")
            nc.vector.tensor_scalar(rbf[:], pf[:], sb_b1[:, ft:ft + 1],
                                    0.0, OP.add, OP.max)
            with nc.allow_low_precision(reason="relu fp8 for fp8 FFN2"):
                nc.vector.tensor_copy(relu8[:, ft, :], rbf[:])
                if relu8lo is not None:
                    nc.gpsimd.tensor_tensor(relu8lo[:, ft, :], rbf[:],
                                            relu8[:, ft, :], OP.subtract)
        return task

    def mk_ffn2_tasks(pr, mt):
        """FFN2 for output tile mt, split into ~1.5us micro-tasks."""
        psl = bass.ts(pr, 512)
        st_ = {}
        # (hl, rhs) matmul units: 32 hi/lo + 16 lo-relu, chunked by 12
        units = ([(0, relu8, j) for j in range(FT // 2)]
                 + [(1, relu8, j) for j in range(FT // 2)])
        if FFN2MM == 3:
            units += [(0, relu8lo, j) for j in range(FT // 2)]

        def c_first():
            w2_s = p_fw.tile([P, 2, FT, P], F8, tag="w2s")
            nc.scalar.dma_start(w2_s[:], w2x_d[:, :, :, bass.ts(mt, P)])
            st_["w"] = w2_s
            st_["po"] = pools_cur["ps"].tile([P, 512], F32, tag="psg",
                                             name="po_f2")

        def mk_chunk(lo_i, hi_i, first):
            def chunk():
                if first:
                    c_first()
                w2_s, po = st_["w"], st_["po"]
                for u in range(lo_i, hi_i):
                    hl, rhs, j = units[u]
                    dr(po[:], w2_s[:, hl, 2 * j:2 * j + 2, :],
                       rhs[:, 2 * j:2 * j + 2, :],
                       start=(u == 0), stop=False)
            return chunk

        def c_last():
            w2_s, po = st_["w"], st_["po"]
            nc.tensor.matmul(po[:], sb_g2d[:, mt, :], fT_bf[:, mt, psl],
                             start=False, stop=True, skip_group_check=True)
            ot = p_fo.tile([P, 512], F32, tag="ot")
            nc.vector.tensor_scalar(ot[:], po[:], 1.0 / (WS_FFN1 * WS_FFN2),
                                    sb_b2[:, mt:mt + 1], OP.mult, OP.add)
            nc.scalar.dma_start(OUT_d[:, mt, psl], ot[:])

        n = len(units)
        step = 12
        tasks = []
        for s in range(0, n, step):
            tasks.append(mk_chunk(s, min(s + step, n), s == 0))
        tasks.append(c_last)
        return tasks

    prev = None
    for qc in range(NQC):
        for t in range(DT):
            for i in range(2):
                exh = attn_scores(t, i, qc)
                if prev is not None:
                    attn_v(*prev)
                prev = (t, i, qc, exh)
                drain(2)
    attn_v(*prev)
    prev = None

    # attention psum pools are done; hand their banks to the FFN tail
    close_pool("apsps")
    close_pool("scps")
    p_pst = open_pool("tailps", 4, space="PSUM")
    pools_cur["ps"] = p_pst
    pools_cur["sc"] = p_pst
    drain(len(bg))

    close_pool("fout")
    close_pool("relubf")
    close_pool("relu")
    close_pool("ffnw")
    close_pool("ln2tmp")
    close_pool("ybp")
    close_pool("xhp")
    close_pool("stage")
    close_pool("expT")
    close_pool("attn")
    close_pool("qkvout")
    close_pool("fTp")
    close_pool("tailps")
    close_pool("psg")
    close_pool("consts")


def _prep_shared(inputs):
    """Host-side weight preprocessing (shared across cores)."""
    f32 = np.float32
    g1 = np.asarray(inputs["g1"], f32)
    beta1 = np.asarray(inputs["beta1"], f32)
    g2 = np.asarray(inputs["g2"], f32)
    beta2 = np.asarray(inputs["beta2"], f32)
    Wq = np.asarray(inputs["Wq"], f32)
    Wk = np.asarray(inputs["Wk"], f32)
    Wv = np.asarray(inputs["Wv"], f32)
    W1 = np.asarray(inputs["W1"], f32)
    W2 = np.asarray(inputs["W2"], f32)

    def fold(Wm, bm):
        Wp = Wm * g1[:, None]
        bp = np.asarray(inputs[bm], f32) + beta1 @ Wm
        return Wp, bp

    Wqp, bqp = fold(Wq, "bq")
    Wkp, bkp = fold(Wk, "bk")
    Wvp, bvp = fold(Wv, "bv")
    W1p = W1 * g2[:, None]
    b1p = np.asarray(inputs["b1"], f32) + beta2 @ W1
    b2p = np.asarray(inputs["b2"], f32) + beta2

    f8 = mybir.dt.np(F8)
    bf = ml_dtypes.bfloat16

    def wtile(Wm, ntile):
        # [K, N] -> [P, ntile, N] with K = ntile*P (partition-major k)
        return np.ascontiguousarray(
            Wm.reshape(ntile, P, Wm.shape[1]).transpose(1, 0, 2))

    def hilo(Wt):
        hi = Wt.astype(f8)
        lo = (Wt - hi.astype(f32)).astype(f8)
        return hi, lo

    wq_t = wtile(WS_QKV * Wqp, DT).astype(f8)
    wk_t = wtile(WS_QKV * Wkp, DT).astype(f8)
    wqk = np.ascontiguousarray(np.stack([wq_t, wk_t], axis=1))
    w1hi, w1lo = hilo(wtile(WS_FFN1 * W1p, DT))
    w1x = np.ascontiguousarray(np.stack([w1hi, w1lo], axis=1))
    w2hi, w2lo = hilo(wtile(WS_FFN2 * W2, FT))
    w2x = np.ascontiguousarray(np.stack([w2hi, w2lo], axis=1))

    g2d = np.zeros((P, DT, P), f32)
    for mt in range(DT):
        np.fill_diagonal(g2d[:, mt, :],
                         WS_FFN1 * WS_FFN2 * g2[mt * P:(mt + 1) * P])

    def btile(bv_, ntile):
        return np.ascontiguousarray(bv_.reshape(ntile, P).T).astype(f32)

    E = np.zeros((16, DT, P), f32)
    for t in range(DT):
        for m in range(P):
            E[2 * t + m // HD, t, m] = 1.0

    return {
        "wqk": wqk, "wv": wtile(WS_V * Wvp, DT).astype(f8),
        "w1x": w1x, "w2x": w2x, "g2d": g2d.astype(bf),
        "bq": btile(bqp, DT), "bk": btile(bkp, DT),
        "bvb": np.ascontiguousarray(
            np.broadcast_to(WS_V * bvp, (P, D))).astype(bf),
        "b1": btile(WS_FFN1 * b1p, FT), "b2": btile(b2p, DT),
        "emat": E.astype(bf),
    }


def _per_core_inputs(inputs, shared):
    x = np.asarray(inputs["x"], np.float32)
    f8 = mybir.dt.np(F8)
    bf = ml_dtypes.bfloat16
    maps = []
    for c in range(NCORES):
        b, hf = c // 2, c % 2
        xTn = x[b].T.reshape(DT, P, S).transpose(1, 0, 2)
        if hf == 1:
            xTn = np.concatenate([xTn[:, :, SQ:], xTn[:, :, :SQ]], axis=2)
        xTn = np.ascontiguousarray(xTn)
        m = dict(shared)
        m["xbf"] = xTn.astype(bf)
        m["xf8"] = xTn.astype(f8)
        m["xsq8"] = (xTn * xTn).astype(f8)
        m["xh"] = np.ascontiguousarray(xTn[:, :, :SQ]).astype(bf)
        maps.append(m)
    return maps


def _get_sharded():
    """Build (once) the nc + jitted shard_map executable."""
    if "sharded" in _CACHE:
        return _CACHE["sharded"]

    import jax
    from jax.sharding import Mesh, PartitionSpec
    from jax.experimental.shard_map import shard_map
    from concourse import bass2jax
    from concourse import mybir as _mybir

    bass2jax.install_neuronx_cc_hook()
    nc = _build_nc()

    partition_name = (nc.partition_id_tensor.name
                      if nc.partition_id_tensor else None)
    in_names, out_names, out_avals, zero_shapes = [], [], [], []
    for alloc in nc.m.functions[0].allocations:
        if not isinstance(alloc, _mybir.MemoryLocationSet):
            continue
        name = alloc.memorylocations[0].name
        if alloc.kind == "ExternalInput":
            if name != partition_name:
                in_names.append(name)
        elif alloc.kind == "ExternalOutput":
            shape = tuple(alloc.tensor_shape)
            dtype = _mybir.dt.np(alloc.dtype)
            out_names.append(name)
            out_avals.append(jax.core.ShapedArray(shape, dtype))
            zero_shapes.append((shape, dtype))
    n_params = len(in_names)
    all_names = in_names + out_names
    if partition_name is not None:
        all_names = all_names + [partition_name]
    donate = tuple(range(n_params, n_params + len(out_names)))

    def _body(*args):
        operands = list(args)
        if partition_name is not None:
            operands.append(bass2jax.partition_id_tensor())
        outs = bass2jax._bass_exec_p.bind(
            *operands,
            out_avals=tuple(out_avals),
            in_names=tuple(all_names),
            out_names=tuple(out_names),
            lowering_input_output_aliases=(),
            sim_require_finite=True,
            sim_require_nnan=True,
            nc=nc,
        )
        return tuple(outs)

    devices = jax.devices()[:NCORES]
    mesh = Mesh(np.asarray(devices), ("core",))
    nin = n_params + len(out_names)
    sharded = jax.jit(
        shard_map(_body, mesh=mesh,
                  in_specs=(PartitionSpec("core"),) * nin,
                  out_specs=(PartitionSpec("core"),) * len(out_names),
                  check_rep=False),
        donate_argnums=donate, keep_unused=True)

    _CACHE["sharded"] = (nc, sharded, in_names, out_names, out_avals,
                         zero_shapes)
    return _CACHE["sharded"]


def _concat_inputs(in_maps):
    _, _, in_names, _, _, zero_shapes = _get_sharded()
    concat_in = [
        np.concatenate([np.asarray(in_maps[c][n]) for c in range(NCORES)],
                       axis=0)
        for n in in_names
    ]
    concat_zeros = [
        np.zeros((NCORES * s[0], *s[1:]), d) for (s, d) in zero_shapes
    ]
    return concat_in, concat_zeros


def _run(in_maps):
    nc, fn, in_names, out_names, out_avals, zero_shapes = _get_sharded()
    concat_in, concat_zeros = _concat_inputs(in_maps)
    outs = fn(*concat_in, *concat_zeros)
    res = []
    for c in range(NCORES):
        res.append({
            name: np.asarray(outs[i]).reshape(NCORES, *out_avals[i].shape)[c]
            for i, name in enumerate(out_names)
        })
    return res


def kernel(**inputs):
    shared = _prep_shared(inputs)
    in_maps = _per_core_inputs(inputs, shared)
    res = _run(in_maps)
    out = np.empty((B, S, D), np.float32)
    for c in range(NCORES):
        b, hf = c // 2, c % 2
        o = res[c]["OUT"]                       # [P, DT, SQ]
        out[b, hf * SQ:(hf + 1) * SQ, :] = o.transpose(2, 1, 0).reshape(SQ, D)
    return out


# revision 88
# speedup vs baseline: 1.0011x; 1.0011x over previous
"""Transformer block (pre-LN MHA + FFN) Trainium2 Bass kernel, fp8 edition.

Data-parallel over 8 cores: core c handles batch b=c//2, sequence half c%2.
Each core computes LN1 + K/V over the batch's FULL 2048 rows, Q/attention/
LN2/FFN over its own 1024 rows (rolled to columns 0:1024 host-side).

All heavy matmuls use fp8e4m3 DoubleRow perf mode (2 stacked 128-deep
contractions per instruction at 0.5 cycles/row):
 - QKV projections / FFN contract pairs of d-tiles of the fp8 activations.
 - Attention scores (64-deep per head) use a zero second stack on the Q
   side (step-sliced view onto a zeroed 9th tile) for 2x.
 - attn@V stacks pairs of k-row tiles of exp(scores) in fp8.
FFN weights are split hi+lo fp8 host-side (error compensation); the relu
activations optionally get the same split (BK_FFN2MM=3).  exp() is scaled
by 1/16 (bias -ln16) so unnormalized attn fits fp8.

LN rstd = exp(-0.5*ln(var+eps)) keeps the ACT engine on a single table
(natural_log_exp_and_others) so the softmax exp stream never reloads.

Emission interleaves FFN/LN2/normalize work for query-chunk pair p under
the ACT-bound attention windows of later chunks (background task queue).

Self-contained: hardcodes shapes B=4, S=2048, D=1024, H=16, FF=4096.
"""

import os

import numpy as np
import ml_dtypes

import concourse.bass as bass
import concourse.bacc as bacc
import concourse.tile as tile
from concourse import mybir

F32 = mybir.dt.float32
BF16 = mybir.dt.bfloat16
F8 = mybir.dt.float8e4
AF = mybir.ActivationFunctionType
OP = mybir.AluOpType
DRM = mybir.MatmulPerfMode.DoubleRow

B, S, D, H, FF = 4, 2048, 1024, 16, 4096
HD = D // H          # 64
P = 128
DT = D // P          # 8  d-tiles
FT = FF // P         # 32 ff-tiles
KT = S // P          # 16 k-row tiles
SQ = S // 2          # 1024 own q columns per core
AQ = 256             # attention q-chunk
NQC = SQ // AQ       # 4
EPS = 1e-5
EXPB = -5.545177444479562   # -ln(256): scale exp so fp8 numerator is safe
NCORES = 8

FFN1MM = int(os.environ.get("BK_FFN1MM", "3"))
FFN2MM = int(os.environ.get("BK_FFN2MM", "3"))
# Host-side weight scales keep fp8e4m3 (max 240, min normal 2^-6) in its
# normal range; descaled at evictions / the denominator stage.
WS_QKV = 32.0        # Wq/Wk scale (q,k descaled at evict)
WS_V = 2.0           # Wv scale (rides into attn numerator; denom stage x2)
WS_FFN1 = 32.0       # W1 scale (relu stage keeps 32x)
WS_FFN2 = 64.0       # W2 scale (final evict divides 32*64)

_CACHE = {}


def _build_nc():
    nc = bacc.Bacc("TRN2", target_bir_lowering=False, debug=False,
                   num_devices=NCORES)

    xbf = nc.dram_tensor("xbf", [P, DT, S], BF16, kind="ExternalInput")
    xf8 = nc.dram_tensor("xf8", [P, DT, S], F8, kind="ExternalInput")
    xsq8 = nc.dram_tensor("xsq8", [P, DT, S], F8, kind="ExternalInput")
    xh = nc.dram_tensor("xh", [P, DT, SQ], BF16, kind="ExternalInput")
    wqk = nc.dram_tensor("wqk", [P, 2, DT, D], F8, kind="ExternalInput")
    wv = nc.dram_tensor("wv", [P, DT, D], F8, kind="ExternalInput")
    w1x = nc.dram_tensor("w1x", [P, 2, DT, FF], F8, kind="ExternalInput")
    w2x = nc.dram_tensor("w2x", [P, 2, FT, D], F8, kind="ExternalInput")
    g2d = nc.dram_tensor("g2d", [P, DT, P], BF16, kind="ExternalInput")
    bq = nc.dram_tensor("bq", [P, DT], F32, kind="ExternalInput")
    bk = nc.dram_tensor("bk", [P, DT], F32, kind="ExternalInput")
    bvb = nc.dram_tensor("bvb", [P, D], BF16, kind="ExternalInput")
    b1 = nc.dram_tensor("b1", [P, FT], F32, kind="ExternalInput")
    b2 = nc.dram_tensor("b2", [P, DT], F32, kind="ExternalInput")
    emat = nc.dram_tensor("emat", [16, DT, P], BF16, kind="ExternalInput")
    OUT = nc.dram_tensor("OUT", [P, DT, SQ], F32, kind="ExternalOutput")

    repeat = int(os.environ.get("BASS_KERNEL_REPEAT", "1"))
    with tile.TileContext(nc) as tc:
        for _ in range(repeat):
            _emit(nc, tc, xbf, xf8, xsq8, xh, wqk, wv, w1x, w2x, g2d,
                  bq, bk, bvb, b1, b2, emat, OUT)
    nc.compile()
    return nc


def _emit(nc, tc, xbf_d, xf8_d, xsq8_d, xh_d, wqk_d, wv_d, w1x_d, w2x_d,
          g2d_d, bq_d, bk_d, bvb_d, b1_d, b2_d, emat_d, OUT_d):
    pools = {}
    pobj = {}

    def open_pool(name, bufs, space="SBUF"):
        cm = tc.tile_pool(name=name, bufs=bufs, space=space)
        pools[name] = cm
        pobj[name] = cm.__enter__()
        return pobj[name]

    def close_pool(name):
        pools.pop(name).__exit__(None, None, None)

    def dr(ps, lhsT, rhs, start, stop):
        nc.tensor.matmul(ps, lhsT, rhs, start=start, stop=stop,
                         perf_mode=DRM, skip_group_check=True)

    p_const = open_pool("consts", 1)
    p_ps = open_pool("psg", 2, space="PSUM")
    p_sc = open_pool("scps", 2, space="PSUM")
    p_aps = open_pool("apsps", 2, space="PSUM")

    # ---- constants ----
    ones8 = p_const.tile([P, 2, P], F8, tag="ones8")
    nc.vector.memset(ones8[:], 1.0)
    ones_bf = p_const.tile([P, P], BF16, tag="ones")
    nc.vector.memset(ones_bf[:], 1.0)
    eps_t = p_const.tile([P, 1], F32, tag="eps")
    nc.vector.memset(eps_t[:], EPS)
    expb_t = p_const.tile([P, 1], F32, tag="expb")
    nc.vector.memset(expb_t[:], EXPB)
    sb_bq = p_const.tile([P, DT], F32, tag="bq")
    nc.sync.dma_start(sb_bq[:], bq_d[:, :])
    sb_bk = p_const.tile([P, DT], F32, tag="bk")
    nc.sync.dma_start(sb_bk[:], bk_d[:, :])
    sb_bvb = p_const.tile([P, D], BF16, tag="bvb")
    nc.sync.dma_start(sb_bvb[:], bvb_d[:, :])
    sb_b1 = p_const.tile([P, FT], F32, tag="b1")
    nc.sync.dma_start(sb_b1[:], b1_d[:, :])
    sb_b2 = p_const.tile([P, DT], F32, tag="b2")
    nc.sync.dma_start(sb_b2[:], b2_d[:, :])
    sb_emat = p_const.tile([16, DT, P], BF16, tag="emat")
    nc.sync.dma_start(sb_emat[:], emat_d[:, :, :])
    sb_g2d = p_const.tile([P, DT, P], BF16, tag="g2d")
    nc.sync.dma_start(sb_g2d[:], g2d_d[:, :, :])

    # ---- persistent activations ----
    p_fT = open_pool("fTp", 1)
    fT_bf = p_fT.tile([P, DT, SQ], BF16, tag="fTbf")
    fT8 = p_fT.tile([P, DT, SQ], F8, tag="fT8")
    fT8lo = (p_fT.tile([P, DT, SQ], F8, tag="fT8lo", name="fT8lo")
             if FFN1MM == 3 else None)

    p_qkv = open_pool("qkvout", 1)
    qT8 = p_qkv.tile([P, DT + 1, SQ], F8, tag="qT8")
    nc.vector.memset(qT8[:, DT, :], 0.0)
    kT8 = p_qkv.tile([P, DT + 1, S], F8, tag="kT8")
    nc.vector.memset(kT8[:, DT, :], 0.0)
    vaug = p_qkv.tile([P, KT, H, HD + 1], F8, tag="vaug")
    nc.vector.memset(vaug[:, :, :, HD:HD + 1], 1.0)

    p_hT = open_pool("hTp", 1)
    hT8 = p_hT.tile([P, DT, S], F8, tag="hT8")

    p_w = open_pool("wslab", 1)
    wv_s = p_w.tile([P, DT, D], F8, tag="wv_s")
    nc.sync.dma_start(wv_s[:], wv_d[:, :, :])

    p_x = open_pool("xp", 1)
    xbf = p_x.tile([P, DT, S], BF16, tag="xbf")
    xf8 = p_x.tile([P, DT, S], F8, tag="xf8")
    xsq8 = p_x.tile([P, DT, S], F8, tag="xsq8")
    p_lt = open_pool("ln1tmp", 1)

    def ln_stats_smalls(ps1, ps2, tmp_pool, sfx, w):
        """psum sums -> (rstd bf16, nsb bf16) tiles of width w."""
        mu = tmp_pool.tile([P, w], F32, tag="mu" + sfx)
        nc.scalar.activation(mu[:], ps1, AF.Copy, bias=0.0, scale=1.0 / D)
        msq = tmp_pool.tile([P, w], F32, tag="msq" + sfx)
        nc.scalar.activation(msq[:], ps2, AF.Copy, bias=0.0, scale=1.0 / D)
        var = tmp_pool.tile([P, w], F32, tag="var" + sfx)
        nc.vector.tensor_mul(var[:], mu[:], mu[:])
        nc.vector.tensor_sub(var[:], msq[:], var[:])
        lnv = tmp_pool.tile([P, w], F32, tag="lnv" + sfx)
        nc.scalar.activation(lnv[:], var[:], AF.Ln, bias=eps_t[:], scale=1.0)
        rstd = tmp_pool.tile([P, w], BF16, tag="rstd" + sfx)
        with nc.allow_low_precision(reason="rstd bf16 feeds bf16 multiplies"):
            nc.scalar.activation(rstd[:], lnv[:], AF.Exp, bias=0.0, scale=-0.5)
        negmu = tmp_pool.tile([P, w], F32, tag="negmu" + sfx)
        nc.scalar.activation(negmu[:], mu[:], AF.Copy, bias=0.0, scale=-1.0)
        nsb = tmp_pool.tile([P, w], BF16, tag="nsb" + sfx)
        nc.vector.tensor_mul(nsb[:], negmu[:], rstd[:])
        return rstd, nsb

    # =========================================================
    # Phase A: LN1 (fp8 DR stats via host x/x^2) + V projection
    # =========================================================
    wqk_s = p_w.tile([P, 2, DT, D], F8, tag="wqk_s")
    nc.sync.dma_start(wqk_s[:], wqk_d[:, :, :, :])

    for sc in range(4):
        ssl = bass.ts(sc, 512)
        nc.sync.dma_start(xbf[:, :, ssl], xbf_d[:, :, ssl])
        nc.sync.dma_start(xf8[:, :, ssl], xf8_d[:, :, ssl])
        nc.sync.dma_start(xsq8[:, :, ssl], xsq8_d[:, :, ssl])

        ps12 = p_sc.tile([P, 2, 512], F32, tag="scps")
        for i in range(4):
            dr(ps12[:, 0, :], ones8[:, :, :], xf8[:, 2 * i:2 * i + 2, ssl],
               start=(i == 0), stop=(i == 3))
        for i in range(4):
            dr(ps12[:, 1, :], ones8[:, :, :], xsq8[:, 2 * i:2 * i + 2, ssl],
               start=(i == 0), stop=(i == 3))
        rstd, nsb = ln_stats_smalls(ps12[:, 0, :], ps12[:, 1, :], p_lt,
                                    "a", 512)
        tmpb = p_lt.tile([P, DT, 512], BF16, tag="tmpb")
        nc.vector.tensor_tensor(
            tmpb[:], xbf[:, :, ssl],
            rstd[:, None, :].to_broadcast((P, DT, 512)), OP.mult)
        with nc.allow_low_precision(reason="hT fp8 matches matmul dtype"):
            nc.vector.tensor_tensor(
                hT8[:, 0:4, ssl], tmpb[:, 0:4, :],
                nsb[:, None, :].to_broadcast((P, 4, 512)), OP.add)
            nc.gpsimd.tensor_tensor(
                hT8[:, 4:DT, ssl], tmpb[:, 4:DT, :],
                nsb[:, None, :].to_broadcast((P, 4, 512)), OP.add)

        # V (Pool evict), K (ACT evict), Q (DVE evict) for this chunk
        for kt in range(4 * sc, 4 * sc + 4):
            for g in range(2):
                pv = p_ps.tile([P, 512], F32, tag="psg")
                for i in range(4):
                    dr(pv[:], hT8[:, 2 * i:2 * i + 2, bass.ts(kt, P)],
                       wv_s[:, 2 * i:2 * i + 2, bass.ts(g, 512)],
                       start=(i == 0), stop=(i == 3))
                with nc.allow_low_precision(reason="v fp8 for fp8 attn"):
                    nc.vector.tensor_tensor(
                        vaug[:, kt, 8 * g:8 * g + 8, 0:HD],
                        pv[:].rearrange("p (h d) -> p h d", d=HD),
                        sb_bvb[:, bass.ts(g, 512)].rearrange(
                            "p (h d) -> p h d", d=HD),
                        OP.add)
        for t in range(DT):
            pk = p_ps.tile([P, 512], F32, tag="psg")
            for i in range(4):
                dr(pk[:], wqk_s[:, 1, 2 * i:2 * i + 2, bass.ts(t, P)],
                   hT8[:, 2 * i:2 * i + 2, ssl],
                   start=(i == 0), stop=(i == 3))
            with nc.allow_low_precision(reason="k fp8 for fp8 attn"):
                nc.scalar.activation(kT8[:, t, ssl], pk[:],
                                     AF.Identity, bias=sb_bk[:, t:t + 1],
                                     scale=1.0 / WS_QKV)
            if sc < 2:
                pq = p_ps.tile([P, 512], F32, tag="psg")
                for i in range(4):
                    dr(pq[:], wqk_s[:, 0, 2 * i:2 * i + 2, bass.ts(t, P)],
                       hT8[:, 2 * i:2 * i + 2, ssl],
                       start=(i == 0), stop=(i == 3))
                with nc.allow_low_precision(reason="q fp8 for fp8 attn"):
                    if t % 2 == 0:
                        nc.scalar.activation(qT8[:, t, ssl], pq[:],
                                             AF.Identity,
                                             bias=sb_bq[:, t:t + 1],
                                             scale=1.0 / WS_QKV)
                    else:
                        nc.vector.tensor_scalar(qT8[:, t, ssl], pq[:],
                                                1.0 / WS_QKV,
                                                sb_bq[:, t:t + 1],
                                                OP.mult, OP.add)

    close_pool("ln1tmp")
    close_pool("xp")
    close_pool("wslab")
    close_pool("hTp")

    # =========================================================
    # Phase C/D/E: attention chunks with interleaved background
    # normalize/LN2/FFN work for earlier chunks.
    # =========================================================
    p_att = open_pool("attn", 1)
    attn8 = p_att.tile([P, DT, SQ], F8, tag="attn8")
    p_ex = open_pool("expT", 2)
    p_st = open_pool("stage", 2)

    p_xh = open_pool("xhp", 1)
    xh = p_xh.tile([P, DT, SQ], BF16, tag="xh")
    for dt_ in range(DT):
        nc.sync.dma_start(xh[:, dt_, :], xh_d[:, dt_, :])
    rpad = p_xh.tile([16, SQ], BF16, tag="rpad")
    nc.vector.memset(rpad[:], 0.0)   # emat matmul reads all rows; keep finite

    p_yb = open_pool("ybp", 1)
    ybf = p_yb.tile([P, DT, SQ], BF16, tag="ybf")
    p_l2 = open_pool("ln2tmp", 1)

    p_fw = open_pool("ffnw", 2)
    p_rl = open_pool("relu", 1)
    relu8 = p_rl.tile([P, FT, 512], F8, tag="relu8")
    relu8lo = (p_rl.tile([P, FT, 512], F8, tag="relu8lo", name="relu8lo")
               if FFN2MM == 3 else None)
    p_rbf = open_pool("relubf", 2)
    p_fo = open_pool("fout", 2)

    bg_hi = []   # normalize / LN2: tiny, unblock downstream
    bg_lo = []   # FFN1/FFN2 in dependency order
    pools_cur = {"ps": p_ps, "sc": p_sc}

    def bg_len():
        return len(bg_hi) + len(bg_lo)

    def drain(n):
        for _ in range(n):
            if bg_hi:
                bg_hi.pop(0)()
            elif bg_lo:
                bg_lo.pop(0)()
            else:
                return

    def attn_scores(t, i, qc):
        """Score burst + exp for head (t, i); returns the per-head ex tile."""
        pb = 64 * i
        qsl = bass.ts(qc, AQ)
        exh = p_ex.tile([P, 16, AQ], F8, tag="expT")
        for g in range(4):
            sc_ps = p_sc.tile([P, 4, AQ], F32, tag="scps")
            for kk in range(4):
                kt = 4 * g + kk
                nc.tensor.matmul(
                    sc_ps[:, kk, :],
                    kT8[pb:pb + 64, t:t + 2, bass.ts(kt, P)],
                    qT8[pb:pb + 64, t:DT + 1:DT - t, qsl],
                    start=(kk % 2 == 0), stop=(kk % 2 == 1),
                    perf_mode=DRM, skip_group_check=True)
            nc.scalar.activation(exh[:, 4 * g:4 * g + 4, :], sc_ps[:], AF.Exp,
                                 bias=expb_t[:], scale=0.125)
        return exh

    def attn_v(t, i, qc, exh):
        """attn@V + evict for head (t, i) using its exp tile."""
        h = 2 * t + i
        pb = 64 * i
        qsl = bass.ts(qc, AQ)
        aps = p_aps.tile([HD + 1, AQ], F32, tag="aps")
        for m in range(8):
            dr(aps[:, :], vaug[:, 2 * m:2 * m + 2, h, :],
               exh[:, 2 * m:2 * m + 2, :],
               start=(m == 0), stop=(m == 7))
        st = p_st.tile([HD, AQ], F8, tag="stage")
        with nc.allow_low_precision(reason="unnormalized attn fp8 (scaled)"):
            nc.vector.tensor_copy(st[:], aps[0:HD, :])
        std = p_st.tile([1, AQ], BF16, tag="staged")
        # denom * WS_V so rpad = 1/(WS_V * den) matches the v scale
        nc.vector.tensor_scalar(std[:], aps[HD:HD + 1, :], WS_V, None,
                                OP.mult)
        rp1 = p_st.tile([1, AQ], BF16, tag="stager")
        with nc.allow_low_precision(reason="softmax denom recip bf16"):
            nc.vector.reciprocal(rp1[:], std[:])
        nc.sync.dma_start(attn8[pb:pb + 64, t, qsl], st[:, :])
        nc.sync.dma_start(rpad[h:h + 1, qsl], rp1[:, :])
        if i == 1:
            bg_hi.append(mk_norm_task(qc, t))
            if t == DT - 1:
                bg_hi.extend(mk_ln2_tasks(qc))
                for ft in range(FT):
                    bg_lo.append(mk_ffn1_task(qc, ft))
                if qc % 2 == 1:
                    for mt in range(DT):
                        bg_lo.extend(mk_ffn2_tasks(qc // 2, mt))

    def mk_norm_task(qc, t):
        """Normalize + residual for d-tile t of chunk qc (heads 2t, 2t+1)."""
        def task():
            qsl = bass.ts(qc, AQ)
            rb = pools_cur["ps"].tile([P, AQ], F32, tag="psg", name="rb")
            nc.tensor.matmul(rb[:], sb_emat[:, t, :], rpad[:, qsl],
                             start=True, stop=True)
            t1 = p_l2.tile([P, AQ], F32, tag="t1")
            nc.vector.tensor_mul(t1[:], attn8[:, t, qsl], rb[:])
            nc.vector.tensor_add(ybf[:, t, qsl], t1[:], xh[:, t, qsl])
        return task

    def mk_ln2_tasks(qc):
        qsl = bass.ts(qc, AQ)
        st_ = {}

        def part_a():
            ysq = p_l2.tile([P, DT, AQ], BF16, tag="scr8a")
            nc.vector.tensor_mul(ysq[:], ybf[:, :, qsl], ybf[:, :, qsl])
            pool = pools_cur["sc"]
            ps12 = pool.tile([P, 2, AQ], F32,
                             tag=("scps" if pool is p_sc else "psg"),
                             name="ln2ps")
            for dt_ in range(DT):
                nc.tensor.matmul(ps12[:, 0, :], ones_bf[:], ybf[:, dt_, qsl],
                                 start=(dt_ == 0), stop=(dt_ == DT - 1))
            for dt_ in range(DT):
                nc.tensor.matmul(ps12[:, 1, :], ones_bf[:], ysq[:, dt_, :],
                                 start=(dt_ == 0), stop=(dt_ == DT - 1))
            st_["ps"] = ps12

        def part_b():
            ps12 = st_["ps"]
            rstd, nsb = ln_stats_smalls(ps12[:, 0, :], ps12[:, 1, :],
                                        p_l2, "b", AQ)
            tmpb = p_l2.tile([P, DT, AQ], BF16, tag="scr8a")
            nc.vector.tensor_tensor(
                tmpb[:], ybf[:, :, qsl],
                rstd[:, None, :].to_broadcast((P, DT, AQ)), OP.mult)
            nc.vector.tensor_tensor(
                fT_bf[:, :, qsl], tmpb[:],
                nsb[:, None, :].to_broadcast((P, DT, AQ)), OP.add)
            with nc.allow_low_precision(reason="f fp8 for fp8 FFN"):
                nc.gpsimd.tensor_tensor(
                    fT8[:, :, qsl], tmpb[:],
                    nsb[:, None, :].to_broadcast((P, DT, AQ)), OP.add)
                if fT8lo is not None:
                    nc.gpsimd.tensor_tensor(fT8lo[:, :, qsl],
                                            fT_bf[:, :, qsl],
                                            fT8[:, :, qsl], OP.subtract)
        return [part_a, part_b]

    def mk_ffn1_task(qc, ft):
        def task():
            qsl = bass.ts(qc, AQ)
            rsl = bass.ts(qc % 2, AQ)     # column range within the pair tile
            w1_s = p_fw.tile([P, 2, DT, P], F8, tag="w1s")
            nc.sync.dma_start(w1_s[:], w1x_d[:, :, :, bass.ts(ft, P)])
            pf = pools_cur["ps"].tile([P, AQ], F32, tag="psg", name="pf")
            mms = [(0, fT8), (1, fT8)]
            if FFN1MM == 3:
                mms.append((0, fT8lo))
            nmm = 0
            tot = 4 * len(mms)
            for hl, rhs in mms:
                for i in range(4):
                    dr(pf[:], w1_s[:, hl, 2 * i:2 * i + 2, :],
                       rhs[:, 2 * i:2 * i + 2, qsl],
                       start=(nmm == 0), stop=(nmm == tot - 1))
                    nmm += 1
            rbf = p_rbf.tile([P, AQ], BF16, tag="rbf")
            nc.vector.tensor_scalar(rbf[:], pf[:], sb_b1[:, ft:ft + 1],
                                    0.0, OP.add, OP.max)
            with nc.allow_low_precision(reason="relu fp8 for fp8 FFN2"):
                nc.vector.tensor_copy(relu8[:, ft, rsl], rbf[:])
                if relu8lo is not None:
                    nc.gpsimd.tensor_tensor(relu8lo[:, ft, rsl], rbf[:],
                                            relu8[:, ft, rsl], OP.subtract)
        return task

    def mk_ffn2_tasks(pr, mt):
        """FFN2 for output tile mt, split into ~1.5us micro-tasks."""
        psl = bass.ts(pr, 512)
        st_ = {}
        # (hl, rhs) matmul units: 32 hi/lo + 16 lo-relu, chunked by 12
        units = ([(0, relu8, j) for j in range(FT // 2)]
                 + [(1, relu8, j) for j in range(FT // 2)])
        if FFN2MM == 3:
            units += [(0, relu8lo, j) for j in range(FT // 2)]

        def c_first():
            w2_s = p_fw.tile([P, 2, FT, P], F8, tag="w2s")
            nc.scalar.dma_start(w2_s[:], w2x_d[:, :, :, bass.ts(mt, P)])
            st_["w"] = w2_s
            st_["po"] = pools_cur["ps"].tile([P, 512], F32, tag="psg",
                                             name="po_f2")

        def mk_chunk(lo_i, hi_i, first):
            def chunk():
                if first:
                    c_first()
                w2_s, po = st_["w"], st_["po"]
                for u in range(lo_i, hi_i):
                    hl, rhs, j = units[u]
                    dr(po[:], w2_s[:, hl, 2 * j:2 * j + 2, :],
                       rhs[:, 2 * j:2 * j + 2, :],
                       start=(u == 0), stop=False)
            return chunk

        def c_last():
            w2_s, po = st_["w"], st_["po"]
            nc.tensor.matmul(po[:], sb_g2d[:, mt, :], fT_bf[:, mt, psl],
                             start=False, stop=True, skip_group_check=True)
            ot = p_fo.tile([P, 512], F32, tag="ot")
            nc.vector.tensor_scalar(ot[:], po[:], 1.0 / (WS_FFN1 * WS_FFN2),
                                    sb_b2[:, mt:mt + 1], OP.mult, OP.add)
            nc.scalar.dma_start(OUT_d[:, mt, psl], ot[:])

        n = len(units)
        step = 12
        tasks = []
        for s in range(0, n, step):
            tasks.append(mk_chunk(s, min(s + step, n), s == 0))
        tasks.append(c_last)
        return tasks

    prev = None
    for qc in range(NQC):
        for t in range(DT):
            for i in range(2):
                exh = attn_scores(t, i, qc)
                if prev is not None:
                    attn_v(*prev)
                prev = (t, i, qc, exh)
                drain(4)
    attn_v(*prev)
    prev = None

    # attention psum pools are done; hand their banks to the FFN tail
    close_pool("apsps")
    close_pool("scps")
    p_pst = open_pool("tailps", 4, space="PSUM")
    pools_cur["ps"] = p_pst
    pools_cur["sc"] = p_pst
    drain(bg_len())

    close_pool("fout")
    close_pool("relubf")
    close_pool("relu")
    close_pool("ffnw")
    close_pool("ln2tmp")
    close_pool("ybp")
    close_pool("xhp")
    close_pool("stage")
    close_pool("expT")
    close_pool("attn")
    close_pool("qkvout")
    close_pool("fTp")
    close_pool("tailps")
    close_pool("psg")
    close_pool("consts")


def _prep_shared(inputs):
    """Host-side weight preprocessing (shared across cores)."""
    f32 = np.float32
    g1 = np.asarray(inputs["g1"], f32)
    beta1 = np.asarray(inputs["beta1"], f32)
    g2 = np.asarray(inputs["g2"], f32)
    beta2 = np.asarray(inputs["beta2"], f32)
    Wq = np.asarray(inputs["Wq"], f32)
    Wk = np.asarray(inputs["Wk"], f32)
    Wv = np.asarray(inputs["Wv"], f32)
    W1 = np.asarray(inputs["W1"], f32)
    W2 = np.asarray(inputs["W2"], f32)

    def fold(Wm, bm):
        Wp = Wm * g1[:, None]
        bp = np.asarray(inputs[bm], f32) + beta1 @ Wm
        return Wp, bp

    Wqp, bqp = fold(Wq, "bq")
    Wkp, bkp = fold(Wk, "bk")
    Wvp, bvp = fold(Wv, "bv")
    W1p = W1 * g2[:, None]
    b1p = np.asarray(inputs["b1"], f32) + beta2 @ W1
    b2p = np.asarray(inputs["b2"], f32) + beta2

    f8 = mybir.dt.np(F8)
    bf = ml_dtypes.bfloat16

    def wtile(Wm, ntile):
        # [K, N] -> [P, ntile, N] with K = ntile*P (partition-major k)
        return np.ascontiguousarray(
            Wm.reshape(ntile, P, Wm.shape[1]).transpose(1, 0, 2))

    def hilo(Wt):
        hi = Wt.astype(f8)
        lo = (Wt - hi.astype(f32)).astype(f8)
        return hi, lo

    wq_t = wtile(WS_QKV * Wqp, DT).astype(f8)
    wk_t = wtile(WS_QKV * Wkp, DT).astype(f8)
    wqk = np.ascontiguousarray(np.stack([wq_t, wk_t], axis=1))
    w1hi, w1lo = hilo(wtile(WS_FFN1 * W1p, DT))
    w1x = np.ascontiguousarray(np.stack([w1hi, w1lo], axis=1))
    w2hi, w2lo = hilo(wtile(WS_FFN2 * W2, FT))
    w2x = np.ascontiguousarray(np.stack([w2hi, w2lo], axis=1))

    g2d = np.zeros((P, DT, P), f32)
    for mt in range(DT):
        np.fill_diagonal(g2d[:, mt, :],
                         WS_FFN1 * WS_FFN2 * g2[mt * P:(mt + 1) * P])

    def btile(bv_, ntile):
        return np.ascontiguousarray(bv_.reshape(ntile, P).T).astype(f32)

    E = np.zeros((16, DT, P), f32)
    for t in range(DT):
        for m in range(P):
            E[2 * t + m // HD, t, m] = 1.0

    return {
        "wqk": wqk, "wv": wtile(WS_V * Wvp, DT).astype(f8),
        "w1x": w1x, "w2x": w2x, "g2d": g2d.astype(bf),
        "bq": btile(bqp, DT), "bk": btile(bkp, DT),
        "bvb": np.ascontiguousarray(
            np.broadcast_to(WS_V * bvp, (P, D))).astype(bf),
        "b1": btile(WS_FFN1 * b1p, FT), "b2": btile(b2p, DT),
        "emat": E.astype(bf),
    }


def _per_core_inputs(inputs, shared):
    x = np.asarray(inputs["x"], np.float32)
    f8 = mybir.dt.np(F8)
    bf = ml_dtypes.bfloat16
    maps = []
    for c in range(NCORES):
        b, hf = c // 2, c % 2
        xTn = x[b].T.reshape(DT, P, S).transpose(1, 0, 2)
        if hf == 1:
            xTn = np.concatenate([xTn[:, :, SQ:], xTn[:, :, :SQ]], axis=2)
        xTn = np.ascontiguousarray(xTn)
        m = dict(shared)
        m["xbf"] = xTn.astype(bf)
        m["xf8"] = xTn.astype(f8)
        m["xsq8"] = (xTn * xTn).astype(f8)
        m["xh"] = np.ascontiguousarray(xTn[:, :, :SQ]).astype(bf)
        maps.append(m)
    return maps


def _get_sharded():
    """Build (once) the nc + jitted shard_map executable."""
    if "sharded" in _CACHE:
        return _CACHE["sharded"]

    import jax
    from jax.sharding import Mesh, PartitionSpec
    from jax.experimental.shard_map import shard_map
    from concourse import bass2jax
    from concourse import mybir as _mybir

    bass2jax.install_neuronx_cc_hook()
    nc = _build_nc()

    partition_name = (nc.partition_id_tensor.name
                      if nc.partition_id_tensor else None)
    in_names, out_names, out_avals, zero_shapes = [], [], [], []
    for alloc in nc.m.functions[0].allocations:
        if not isinstance(alloc, _mybir.MemoryLocationSet):
            continue
        name = alloc.memorylocations[0].name
        if alloc.kind == "ExternalInput":
            if name != partition_name:
                in_names.append(name)
        elif alloc.kind == "ExternalOutput":
            shape = tuple(alloc.tensor_shape)
            dtype = _mybir.dt.np(alloc.dtype)
            out_names.append(name)
            out_avals.append(jax.core.ShapedArray(shape, dtype))
            zero_shapes.append((shape, dtype))
    n_params = len(in_names)
    all_names = in_names + out_names
    if partition_name is not None:
        all_names = all_names + [partition_name]
    donate = tuple(range(n_params, n_params + len(out_names)))

    def _body(*args):
        operands = list(args)
        if partition_name is not None:
            operands.append(bass2jax.partition_id_tensor())
        outs = bass2jax._bass_exec_p.bind(
            *operands,
            out_avals=tuple(out_avals),
            in_names=tuple(all_names),
            out_names=tuple(out_names),
            lowering_input_output_aliases=(),
            sim_require_finite=True,
            sim_require_nnan=True,
            nc=nc,
        )
        return tuple(outs)

    devices = jax.devices()[:NCORES]
    mesh = Mesh(np.asarray(devices), ("core",))
    nin = n_params + len(out_names)
    sharded = jax.jit(
        shard_map(_body, mesh=mesh,
                  in_specs=(PartitionSpec("core"),) * nin,
                  out_specs=(PartitionSpec("core"),) * len(out_names),
                  check_rep=False),
        donate_argnums=donate, keep_unused=True)

    _CACHE["sharded"] = (nc, sharded, in_names, out_names, out_avals,
                         zero_shapes)
    return _CACHE["sharded"]


def _concat_inputs(in_maps):
    _, _, in_names, _, _, zero_shapes = _get_sharded()
    concat_in = [
        np.concatenate([np.asarray(in_maps[c][n]) for c in range(NCORES)],
                       axis=0)
        for n in in_names
    ]
    concat_zeros = [
        np.zeros((NCORES * s[0], *s[1:]), d) for (s, d) in zero_shapes
    ]
    return concat_in, concat_zeros


def _run(in_maps):
    nc, fn, in_names, out_names, out_avals, zero_shapes = _get_sharded()
    concat_in, concat_zeros = _concat_inputs(in_maps)
    outs = fn(*concat_in, *concat_zeros)
    res = []
    for c in range(NCORES):
        res.append({
            name: np.asarray(outs[i]).reshape(NCORES, *out_avals[i].shape)[c]
            for i, name in enumerate(out_names)
        })
    return res


def kernel(**inputs):
    shared = _prep_shared(inputs)
    in_maps = _per_core_inputs(inputs, shared)
    res = _run(in_maps)
    out = np.empty((B, S, D), np.float32)
    for c in range(NCORES):
        b, hf = c // 2, c % 2
        o = res[c]["OUT"]                       # [P, DT, SQ]
        out[b, hf * SQ:(hf + 1) * SQ, :] = o.transpose(2, 1, 0).reshape(SQ, D)
    return out


# revision 94
# speedup vs baseline: 1.1161x; 1.1149x over previous
"""Transformer block (pre-LN MHA + FFN) Trainium2 Bass kernel, fp8 edition.

Data-parallel over 8 cores: core c handles batch b=c//2, sequence half c%2.
Each core computes LN1 + K/V over the batch's FULL 2048 rows, Q/attention/
LN2/FFN over its own 1024 rows (rolled to columns 0:1024 host-side).

All heavy matmuls use fp8e4m3 DoubleRow perf mode (2 stacked 128-deep
contractions per instruction at 0.5 cycles/row):
 - QKV projections / FFN contract pairs of d-tiles of the fp8 activations.
 - Attention scores (64-deep per head) use a zero second stack on the Q
   side (step-sliced view onto a zeroed 9th tile) for 2x.
 - attn@V stacks pairs of k-row tiles of exp(scores) in fp8.
FFN weights are split hi+lo fp8 host-side (error compensation); the relu
activations optionally get the same split (BK_FFN2MM=3).  exp() is scaled
by 1/16 (bias -ln16) so unnormalized attn fits fp8.

LN rstd = exp(-0.5*ln(var+eps)) keeps the ACT engine on a single table
(natural_log_exp_and_others) so the softmax exp stream never reloads.

Emission interleaves FFN/LN2/normalize work for query-chunk pair p under
the ACT-bound attention windows of later chunks (background task queue).

Self-contained: hardcodes shapes B=4, S=2048, D=1024, H=16, FF=4096.
"""

import os

import numpy as np
import ml_dtypes

import concourse.bass as bass
import concourse.bacc as bacc
import concourse.tile as tile
from concourse import mybir

F32 = mybir.dt.float32
BF16 = mybir.dt.bfloat16
F8 = mybir.dt.float8e4
AF = mybir.ActivationFunctionType
OP = mybir.AluOpType
DRM = mybir.MatmulPerfMode.DoubleRow

B, S, D, H, FF = 4, 2048, 1024, 16, 4096
HD = D // H          # 64
P = 128
DT = D // P          # 8  d-tiles
FT = FF // P         # 32 ff-tiles
KT = S // P          # 16 k-row tiles
SQ = S // 2          # 1024 own q columns per core
AQ = 256             # attention q-chunk
NQC = SQ // AQ       # 4
EPS = 1e-5
EXPB = -5.545177444479562   # -ln(256): scale exp so fp8 numerator is safe
NCORES = 8

FFN1MM = int(os.environ.get("BK_FFN1MM", "3"))
FFN2MM = int(os.environ.get("BK_FFN2MM", "3"))
# Host-side weight scales keep fp8e4m3 (max 240, min normal 2^-6) in its
# normal range; descaled at evictions / the denominator stage.
WS_QKV = 32.0        # Wq/Wk scale (q,k descaled at evict)
WS_V = 2.0           # Wv scale (rides into attn numerator; denom stage x2)
WS_FFN1 = 32.0       # W1 scale (relu stage keeps 32x)
WS_FFN2 = 64.0       # W2 scale (final evict divides 32*64)

_CACHE = {}


def _build_nc():
    nc = bacc.Bacc("TRN2", target_bir_lowering=False, debug=False,
                   num_devices=NCORES)

    xbf = nc.dram_tensor("xbf", [P, DT, S], BF16, kind="ExternalInput")
    xf8 = nc.dram_tensor("xf8", [P, DT, S], F8, kind="ExternalInput")
    xsq8 = nc.dram_tensor("xsq8", [P, DT, S], F8, kind="ExternalInput")
    xh = nc.dram_tensor("xh", [P, DT, SQ], BF16, kind="ExternalInput")
    wqk = nc.dram_tensor("wqk", [P, 2, DT, D], F8, kind="ExternalInput")
    wv = nc.dram_tensor("wv", [P, DT, D], F8, kind="ExternalInput")
    w1x = nc.dram_tensor("w1x", [P, FT, 2, DT, P], F8, kind="ExternalInput")
    w2x = nc.dram_tensor("w2x", [P, DT, 2, FT, P], F8, kind="ExternalInput")
    g2d = nc.dram_tensor("g2d", [P, DT, P], BF16, kind="ExternalInput")
    bq = nc.dram_tensor("bq", [P, DT], F32, kind="ExternalInput")
    bk = nc.dram_tensor("bk", [P, DT], F32, kind="ExternalInput")
    bvb = nc.dram_tensor("bvb", [P, D], BF16, kind="ExternalInput")
    b1 = nc.dram_tensor("b1", [P, FT], F32, kind="ExternalInput")
    b2 = nc.dram_tensor("b2", [P, DT], F32, kind="ExternalInput")
    emat = nc.dram_tensor("emat", [16, DT, P], BF16, kind="ExternalInput")
    OUT = nc.dram_tensor("OUT", [P, DT, SQ], F32, kind="ExternalOutput")

    repeat = int(os.environ.get("BASS_KERNEL_REPEAT", "1"))
    with tile.TileContext(nc) as tc:
        for _ in range(repeat):
            _emit(nc, tc, xbf, xf8, xsq8, xh, wqk, wv, w1x, w2x, g2d,
                  bq, bk, bvb, b1, b2, emat, OUT)
    nc.compile()
    return nc


def _emit(nc, tc, xbf_d, xf8_d, xsq8_d, xh_d, wqk_d, wv_d, w1x_d, w2x_d,
          g2d_d, bq_d, bk_d, bvb_d, b1_d, b2_d, emat_d, OUT_d):
    pools = {}
    pobj = {}

    def open_pool(name, bufs, space="SBUF"):
        cm = tc.tile_pool(name=name, bufs=bufs, space=space)
        pools[name] = cm
        pobj[name] = cm.__enter__()
        return pobj[name]

    def close_pool(name):
        pools.pop(name).__exit__(None, None, None)

    def dr(ps, lhsT, rhs, start, stop):
        nc.tensor.matmul(ps, lhsT, rhs, start=start, stop=stop,
                         perf_mode=DRM, skip_group_check=True)

    p_const = open_pool("consts", 1)
    p_ps = open_pool("psg", 2, space="PSUM")
    p_sc = open_pool("scps", 2, space="PSUM")
    p_aps = open_pool("apsps", 2, space="PSUM")

    # ---- constants ----
    ones8 = p_const.tile([P, 2, P], F8, tag="ones8")
    nc.vector.memset(ones8[:], 1.0)
    ones_bf = p_const.tile([P, P], BF16, tag="ones")
    nc.vector.memset(ones_bf[:], 1.0)
    eps_t = p_const.tile([P, 1], F32, tag="eps")
    nc.vector.memset(eps_t[:], EPS)
    expb_t = p_const.tile([P, 1], F32, tag="expb")
    nc.vector.memset(expb_t[:], EXPB)
    sb_bq = p_const.tile([P, DT], F32, tag="bq")
    nc.sync.dma_start(sb_bq[:], bq_d[:, :])
    sb_bk = p_const.tile([P, DT], F32, tag="bk")
    nc.sync.dma_start(sb_bk[:], bk_d[:, :])
    sb_bvb = p_const.tile([P, D], BF16, tag="bvb")
    nc.sync.dma_start(sb_bvb[:], bvb_d[:, :])
    sb_b1 = p_const.tile([P, FT], F32, tag="b1")
    nc.sync.dma_start(sb_b1[:], b1_d[:, :])
    sb_b2 = p_const.tile([P, DT], F32, tag="b2")
    nc.sync.dma_start(sb_b2[:], b2_d[:, :])
    sb_emat = p_const.tile([16, DT, P], BF16, tag="emat")
    nc.sync.dma_start(sb_emat[:], emat_d[:, :, :])
    sb_g2d = p_const.tile([P, DT, P], BF16, tag="g2d")
    nc.sync.dma_start(sb_g2d[:], g2d_d[:, :, :])

    # ---- persistent activations ----
    p_fT = open_pool("fTp", 1)
    fT_bf = p_fT.tile([P, DT, SQ], BF16, tag="fTbf")
    fT8 = p_fT.tile([P, DT, SQ], F8, tag="fT8")
    fT8lo = (p_fT.tile([P, DT, SQ], F8, tag="fT8lo", name="fT8lo")
             if FFN1MM == 3 else None)

    p_qkv = open_pool("qkvout", 1)
    qT8 = p_qkv.tile([P, DT + 1, SQ], F8, tag="qT8")
    nc.vector.memset(qT8[:, DT, :], 0.0)
    kT8 = p_qkv.tile([P, DT + 1, S], F8, tag="kT8")
    nc.vector.memset(kT8[:, DT, :], 0.0)
    vaug = p_qkv.tile([P, KT, H, HD + 1], F8, tag="vaug")
    nc.vector.memset(vaug[:, :, :, HD:HD + 1], 1.0)

    p_hT = open_pool("hTp", 1)
    hT8 = p_hT.tile([P, DT, S], F8, tag="hT8")

    p_w = open_pool("wslab", 1)
    wv_s = p_w.tile([P, DT, D], F8, tag="wv_s")

    p_x = open_pool("xp", 1)
    xbf = p_x.tile([P, DT, S], BF16, tag="xbf")
    xf8 = p_x.tile([P, DT, S], F8, tag="xf8")
    xsq8 = p_x.tile([P, DT, S], F8, tag="xsq8")
    p_lt = open_pool("ln1tmp", 1)

    def ln_stats_smalls(ps1, ps2, tmp_pool, sfx, w):
        """psum sums -> (rstd bf16, nsb bf16) tiles of width w."""
        mu = tmp_pool.tile([P, w], F32, tag="mu" + sfx)
        nc.scalar.activation(mu[:], ps1, AF.Copy, bias=0.0, scale=1.0 / D)
        msq = tmp_pool.tile([P, w], F32, tag="msq" + sfx)
        nc.scalar.activation(msq[:], ps2, AF.Copy, bias=0.0, scale=1.0 / D)
        var = tmp_pool.tile([P, w], F32, tag="var" + sfx)
        nc.vector.tensor_mul(var[:], mu[:], mu[:])
        nc.vector.tensor_sub(var[:], msq[:], var[:])
        lnv = tmp_pool.tile([P, w], F32, tag="lnv" + sfx)
        nc.scalar.activation(lnv[:], var[:], AF.Ln, bias=eps_t[:], scale=1.0)
        rstd = tmp_pool.tile([P, w], BF16, tag="rstd" + sfx)
        with nc.allow_low_precision(reason="rstd bf16 feeds bf16 multiplies"):
            nc.scalar.activation(rstd[:], lnv[:], AF.Exp, bias=0.0, scale=-0.5)
        negmu = tmp_pool.tile([P, w], F32, tag="negmu" + sfx)
        nc.scalar.activation(negmu[:], mu[:], AF.Copy, bias=0.0, scale=-1.0)
        nsb = tmp_pool.tile([P, w], BF16, tag="nsb" + sfx)
        nc.vector.tensor_mul(nsb[:], negmu[:], rstd[:])
        return rstd, nsb

    # =========================================================
    # Phase A: LN1 (fp8 DR stats via host x/x^2) + V projection
    # =========================================================
    wqk_s = p_w.tile([P, 2, DT, D], F8, tag="wqk_s")

    for sc in range(4):
        ssl = bass.ts(sc, 512)
        nc.sync.dma_start(xbf[:, :, ssl], xbf_d[:, :, ssl])
        nc.sync.dma_start(xf8[:, :, ssl], xf8_d[:, :, ssl])
        nc.sync.dma_start(xsq8[:, :, ssl], xsq8_d[:, :, ssl])
        if sc == 0:
            # weights after the first LN1 chunk's inputs on the queue
            nc.sync.dma_start(wv_s[:], wv_d[:, :, :])
            nc.sync.dma_start(wqk_s[:], wqk_d[:, :, :, :])

        ps12 = p_sc.tile([P, 2, 512], F32, tag="scps")
        for i in range(4):
            dr(ps12[:, 0, :], ones8[:, :, :], xf8[:, 2 * i:2 * i + 2, ssl],
               start=(i == 0), stop=(i == 3))
        for i in range(4):
            dr(ps12[:, 1, :], ones8[:, :, :], xsq8[:, 2 * i:2 * i + 2, ssl],
               start=(i == 0), stop=(i == 3))
        rstd, nsb = ln_stats_smalls(ps12[:, 0, :], ps12[:, 1, :], p_lt,
                                    "a", 512)
        tmpb = p_lt.tile([P, DT, 512], BF16, tag="tmpb")
        nc.vector.tensor_tensor(
            tmpb[:], xbf[:, :, ssl],
            rstd[:, None, :].to_broadcast((P, DT, 512)), OP.mult)
        with nc.allow_low_precision(reason="hT fp8 matches matmul dtype"):
            nc.vector.tensor_tensor(
                hT8[:, 0:4, ssl], tmpb[:, 0:4, :],
                nsb[:, None, :].to_broadcast((P, 4, 512)), OP.add)
            nc.gpsimd.tensor_tensor(
                hT8[:, 4:DT, ssl], tmpb[:, 4:DT, :],
                nsb[:, None, :].to_broadcast((P, 4, 512)), OP.add)

        # V (Pool evict), K (ACT evict), Q (DVE evict) for this chunk
        for kt in range(4 * sc, 4 * sc + 4):
            for g in range(2):
                pv = p_ps.tile([P, 512], F32, tag="psg")
                for i in range(4):
                    dr(pv[:], hT8[:, 2 * i:2 * i + 2, bass.ts(kt, P)],
                       wv_s[:, 2 * i:2 * i + 2, bass.ts(g, 512)],
                       start=(i == 0), stop=(i == 3))
                with nc.allow_low_precision(reason="v fp8 for fp8 attn"):
                    nc.vector.tensor_tensor(
                        vaug[:, kt, 8 * g:8 * g + 8, 0:HD],
                        pv[:].rearrange("p (h d) -> p h d", d=HD),
                        sb_bvb[:, bass.ts(g, 512)].rearrange(
                            "p (h d) -> p h d", d=HD),
                        OP.add)
        for t in range(DT):
            pk = p_ps.tile([P, 512], F32, tag="psg")
            for i in range(4):
                dr(pk[:], wqk_s[:, 1, 2 * i:2 * i + 2, bass.ts(t, P)],
                   hT8[:, 2 * i:2 * i + 2, ssl],
                   start=(i == 0), stop=(i == 3))
            with nc.allow_low_precision(reason="k fp8 for fp8 attn"):
                nc.scalar.activation(kT8[:, t, ssl], pk[:],
                                     AF.Identity, bias=sb_bk[:, t:t + 1],
                                     scale=1.0 / WS_QKV)
            if sc < 2:
                pq = p_ps.tile([P, 512], F32, tag="psg")
                for i in range(4):
                    dr(pq[:], wqk_s[:, 0, 2 * i:2 * i + 2, bass.ts(t, P)],
                       hT8[:, 2 * i:2 * i + 2, ssl],
                       start=(i == 0), stop=(i == 3))
                with nc.allow_low_precision(reason="q fp8 for fp8 attn"):
                    if t % 2 == 0:
                        nc.scalar.activation(qT8[:, t, ssl], pq[:],
                                             AF.Identity,
                                             bias=sb_bq[:, t:t + 1],
                                             scale=1.0 / WS_QKV)
                    else:
                        nc.vector.tensor_scalar(qT8[:, t, ssl], pq[:],
                                                1.0 / WS_QKV,
                                                sb_bq[:, t:t + 1],
                                                OP.mult, OP.add)

    close_pool("ln1tmp")
    close_pool("xp")
    close_pool("wslab")
    close_pool("hTp")

    # =========================================================
    # Phase C/D/E: attention chunks with interleaved background
    # normalize/LN2/FFN work for earlier chunks.
    # =========================================================
    p_att = open_pool("attn", 1)
    attn8 = p_att.tile([P, DT, SQ], F8, tag="attn8")
    p_ex = open_pool("expT", 2)
    p_st = open_pool("stage", 2)

    p_xh = open_pool("xhp", 1)
    xh = p_xh.tile([P, DT, SQ], BF16, tag="xh")
    for dt_ in range(DT):
        nc.sync.dma_start(xh[:, dt_, :], xh_d[:, dt_, :])
    rpad = p_xh.tile([16, SQ], BF16, tag="rpad")
    nc.vector.memset(rpad[:], 0.0)   # emat matmul reads all rows; keep finite

    p_yb = open_pool("ybp", 1)
    ybf = p_yb.tile([P, DT, SQ], BF16, tag="ybf")
    p_l2 = open_pool("ln2tmp", 1)

    p_fw = open_pool("ffnw", 2)
    p_rl = open_pool("relu", 1)
    relu8 = p_rl.tile([P, FT, 512], F8, tag="relu8")
    relu8lo = (p_rl.tile([P, FT, 512], F8, tag="relu8lo", name="relu8lo")
               if FFN2MM == 3 else None)
    p_rbf = open_pool("relubf", 2)
    p_fo = open_pool("fout", 2)

    bg_hi = []   # normalize / LN2: tiny, unblock downstream
    bg_lo = []   # FFN1/FFN2 in dependency order
    pools_cur = {"ps": p_ps, "sc": p_sc}

    def bg_len():
        return len(bg_hi) + len(bg_lo)

    def drain(n):
        for _ in range(n):
            if bg_hi:
                bg_hi.pop(0)()
            elif bg_lo:
                bg_lo.pop(0)()
            else:
                return

    def attn_scores(t, i, qc):
        """Score burst + exp for head (t, i); returns the per-head ex tile."""
        pb = 64 * i
        qsl = bass.ts(qc, AQ)
        exh = p_ex.tile([P, 16, AQ], F8, tag="expT")
        for g in range(4):
            sc_ps = p_sc.tile([P, 4, AQ], F32, tag="scps")
            for kk in range(4):
                kt = 4 * g + kk
                nc.tensor.matmul(
                    sc_ps[:, kk, :],
                    kT8[pb:pb + 64, t:t + 2, bass.ts(kt, P)],
                    qT8[pb:pb + 64, t:DT + 1:DT - t, qsl],
                    start=(kk % 2 == 0), stop=(kk % 2 == 1),
                    perf_mode=DRM, skip_group_check=True)
            nc.scalar.activation(exh[:, 4 * g:4 * g + 4, :], sc_ps[:], AF.Exp,
                                 bias=expb_t[:], scale=0.125)
        return exh

    def attn_v(t, i, qc, exh):
        """attn@V + evict for head (t, i) using its exp tile."""
        h = 2 * t + i
        pb = 64 * i
        qsl = bass.ts(qc, AQ)
        aps = p_aps.tile([HD + 1, AQ], F32, tag="aps")
        for m in range(8):
            dr(aps[:, :], vaug[:, 2 * m:2 * m + 2, h, :],
               exh[:, 2 * m:2 * m + 2, :],
               start=(m == 0), stop=(m == 7))
        st = p_st.tile([HD, AQ], F8, tag="stage")
        with nc.allow_low_precision(reason="unnormalized attn fp8 (scaled)"):
            nc.vector.tensor_copy(st[:], aps[0:HD, :])
        std = p_st.tile([1, AQ], BF16, tag="staged")
        # denom * WS_V so rpad = 1/(WS_V * den) matches the v scale
        nc.vector.tensor_scalar(std[:], aps[HD:HD + 1, :], WS_V, None,
                                OP.mult)
        rp1 = p_st.tile([1, AQ], BF16, tag="stager")
        with nc.allow_low_precision(reason="softmax denom recip bf16"):
            nc.vector.reciprocal(rp1[:], std[:])
        nc.sync.dma_start(attn8[pb:pb + 64, t, qsl], st[:, :])
        nc.sync.dma_start(rpad[h:h + 1, qsl], rp1[:, :])
        if i == 1:
            bg_hi.append(mk_norm_task(qc, t))
            if t == DT - 1:
                bg_hi.extend(mk_ln2_tasks(qc))
                for ft in range(FT):
                    bg_lo.append(mk_ffn1_task(qc, ft))
                if qc % 2 == 1:
                    for mt in range(DT):
                        bg_lo.extend(mk_ffn2_tasks(qc // 2, mt))

    def mk_norm_task(qc, t):
        """Normalize + residual for d-tile t of chunk qc (heads 2t, 2t+1)."""
        def task():
            qsl = bass.ts(qc, AQ)
            rb = pools_cur["ps"].tile([P, AQ], F32, tag="psg", name="rb")
            nc.tensor.matmul(rb[:], sb_emat[:, t, :], rpad[:, qsl],
                             start=True, stop=True)
            t1 = p_l2.tile([P, AQ], F32, tag="t1")
            nc.vector.tensor_mul(t1[:], attn8[:, t, qsl], rb[:])
            nc.vector.tensor_add(ybf[:, t, qsl], t1[:], xh[:, t, qsl])
        return task

    def mk_ln2_tasks(qc):
        qsl = bass.ts(qc, AQ)
        st_ = {}

        def part_a():
            ysq = p_l2.tile([P, DT, AQ], BF16, tag="scr8a")
            nc.vector.tensor_mul(ysq[:], ybf[:, :, qsl], ybf[:, :, qsl])
            pool = pools_cur["sc"]
            ps12 = pool.tile([P, 2, AQ], F32,
                             tag=("scps" if pool is p_sc else "psg"),
                             name="ln2ps")
            for dt_ in range(DT):
                nc.tensor.matmul(ps12[:, 0, :], ones_bf[:], ybf[:, dt_, qsl],
                                 start=(dt_ == 0), stop=(dt_ == DT - 1))
            for dt_ in range(DT):
                nc.tensor.matmul(ps12[:, 1, :], ones_bf[:], ysq[:, dt_, :],
                                 start=(dt_ == 0), stop=(dt_ == DT - 1))
            st_["ps"] = ps12

        def part_b():
            ps12 = st_["ps"]
            rstd, nsb = ln_stats_smalls(ps12[:, 0, :], ps12[:, 1, :],
                                        p_l2, "b", AQ)
            tmpb = p_l2.tile([P, DT, AQ], BF16, tag="scr8a")
            nc.vector.tensor_tensor(
                tmpb[:], ybf[:, :, qsl],
                rstd[:, None, :].to_broadcast((P, DT, AQ)), OP.mult)
            nc.vector.tensor_tensor(
                fT_bf[:, :, qsl], tmpb[:],
                nsb[:, None, :].to_broadcast((P, DT, AQ)), OP.add)
            with nc.allow_low_precision(reason="f fp8 for fp8 FFN"):
                nc.gpsimd.tensor_tensor(
                    fT8[:, :, qsl], tmpb[:],
                    nsb[:, None, :].to_broadcast((P, DT, AQ)), OP.add)
                if fT8lo is not None:
                    nc.gpsimd.tensor_tensor(fT8lo[:, :, qsl],
                                            fT_bf[:, :, qsl],
                                            fT8[:, :, qsl], OP.subtract)
        return [part_a, part_b]

    def mk_ffn1_task(qc, ft):
        def task():
            qsl = bass.ts(qc, AQ)
            rsl = bass.ts(qc % 2, AQ)     # column range within the pair tile
            w1_s = p_fw.tile([P, 2, DT, P], F8, tag="w1s")
            nc.sync.dma_start(w1_s[:], w1x_d[:, ft, :, :, :])
            pf = pools_cur["ps"].tile([P, AQ], F32, tag="psg", name="pf")
            mms = [(0, fT8), (1, fT8)]
            if FFN1MM == 3:
                mms.append((0, fT8lo))
            nmm = 0
            tot = 4 * len(mms)
            for hl, rhs in mms:
                for i in range(4):
                    dr(pf[:], w1_s[:, hl, 2 * i:2 * i + 2, :],
                       rhs[:, 2 * i:2 * i + 2, qsl],
                       start=(nmm == 0), stop=(nmm == tot - 1))
                    nmm += 1
            rbf = p_rbf.tile([P, AQ], BF16, tag="rbf")
            nc.vector.tensor_scalar(rbf[:], pf[:], sb_b1[:, ft:ft + 1],
                                    0.0, OP.add, OP.max)
            with nc.allow_low_precision(reason="relu fp8 for fp8 FFN2"):
                nc.vector.tensor_copy(relu8[:, ft, rsl], rbf[:])
                if relu8lo is not None:
                    nc.gpsimd.tensor_tensor(relu8lo[:, ft, rsl], rbf[:],
                                            relu8[:, ft, rsl], OP.subtract)
        return task

    def mk_ffn2_tasks(pr, mt):
        """FFN2 for output tile mt, split into ~1.5us micro-tasks."""
        psl = bass.ts(pr, 512)
        st_ = {}
        # (hl, rhs) matmul units: 32 hi/lo + 16 lo-relu, chunked by 12
        units = ([(0, relu8, j) for j in range(FT // 2)]
                 + [(1, relu8, j) for j in range(FT // 2)])
        if FFN2MM == 3:
            units += [(0, relu8lo, j) for j in range(FT // 2)]

        def c_first():
            w2_s = p_fw.tile([P, 2, FT, P], F8, tag="w2s")
            nc.scalar.dma_start(w2_s[:], w2x_d[:, mt, :, :, :])
            st_["w"] = w2_s
            st_["po"] = pools_cur["ps"].tile([P, 512], F32, tag="psg",
                                             name="po_f2")

        def mk_chunk(lo_i, hi_i, first):
            def chunk():
                if first:
                    c_first()
                w2_s, po = st_["w"], st_["po"]
                for u in range(lo_i, hi_i):
                    hl, rhs, j = units[u]
                    dr(po[:], w2_s[:, hl, 2 * j:2 * j + 2, :],
                       rhs[:, 2 * j:2 * j + 2, :],
                       start=(u == 0), stop=False)
            return chunk

        def c_last():
            w2_s, po = st_["w"], st_["po"]
            nc.tensor.matmul(po[:], sb_g2d[:, mt, :], fT_bf[:, mt, psl],
                             start=False, stop=True, skip_group_check=True)
            ot = p_fo.tile([P, 512], F32, tag="ot")
            nc.vector.tensor_scalar(ot[:], po[:], 1.0 / (WS_FFN1 * WS_FFN2),
                                    sb_b2[:, mt:mt + 1], OP.mult, OP.add)
            nc.scalar.dma_start(OUT_d[:, mt, psl], ot[:])

        n = len(units)
        step = 12
        tasks = []
        for s in range(0, n, step):
            tasks.append(mk_chunk(s, min(s + step, n), s == 0))
        tasks.append(c_last)
        return tasks

    prev = None
    for qc in range(NQC):
        for t in range(DT):
            for i in range(2):
                exh = attn_scores(t, i, qc)
                if prev is not None:
                    attn_v(*prev)
                prev = (t, i, qc, exh)
                drain(4)
    attn_v(*prev)
    prev = None

    # attention psum pools are done; hand their banks to the FFN tail
    close_pool("apsps")
    close_pool("scps")
    p_pst = open_pool("tailps", 4, space="PSUM")
    pools_cur["ps"] = p_pst
    pools_cur["sc"] = p_pst
    drain(bg_len())

    close_pool("fout")
    close_pool("relubf")
    close_pool("relu")
    close_pool("ffnw")
    close_pool("ln2tmp")
    close_pool("ybp")
    close_pool("xhp")
    close_pool("stage")
    close_pool("expT")
    close_pool("attn")
    close_pool("qkvout")
    close_pool("fTp")
    close_pool("tailps")
    close_pool("psg")
    close_pool("consts")


def _prep_shared(inputs):
    """Host-side weight preprocessing (shared across cores)."""
    f32 = np.float32
    g1 = np.asarray(inputs["g1"], f32)
    beta1 = np.asarray(inputs["beta1"], f32)
    g2 = np.asarray(inputs["g2"], f32)
    beta2 = np.asarray(inputs["beta2"], f32)
    Wq = np.asarray(inputs["Wq"], f32)
    Wk = np.asarray(inputs["Wk"], f32)
    Wv = np.asarray(inputs["Wv"], f32)
    W1 = np.asarray(inputs["W1"], f32)
    W2 = np.asarray(inputs["W2"], f32)

    def fold(Wm, bm):
        Wp = Wm * g1[:, None]
        bp = np.asarray(inputs[bm], f32) + beta1 @ Wm
        return Wp, bp

    Wqp, bqp = fold(Wq, "bq")
    Wkp, bkp = fold(Wk, "bk")
    Wvp, bvp = fold(Wv, "bv")
    W1p = W1 * g2[:, None]
    b1p = np.asarray(inputs["b1"], f32) + beta2 @ W1
    b2p = np.asarray(inputs["b2"], f32) + beta2

    f8 = mybir.dt.np(F8)
    bf = ml_dtypes.bfloat16

    def wtile(Wm, ntile):
        # [K, N] -> [P, ntile, N] with K = ntile*P (partition-major k)
        return np.ascontiguousarray(
            Wm.reshape(ntile, P, Wm.shape[1]).transpose(1, 0, 2))

    def hilo(Wt):
        hi = Wt.astype(f8)
        lo = (Wt - hi.astype(f32)).astype(f8)
        return hi, lo

    wq_t = wtile(WS_QKV * Wqp, DT).astype(f8)
    wk_t = wtile(WS_QKV * Wkp, DT).astype(f8)
    wqk = np.ascontiguousarray(np.stack([wq_t, wk_t], axis=1))
    w1hi, w1lo = hilo(wtile(WS_FFN1 * W1p, DT))
    # slab-contiguous: [P, FT, 2, DT, 128] so one ft slab is one 2KB run
    w1x = np.ascontiguousarray(
        np.stack([w1hi, w1lo], axis=1).reshape(P, 2, DT, FT, P)
        .transpose(0, 3, 1, 2, 4))
    w2hi, w2lo = hilo(wtile(WS_FFN2 * W2, FT))
    # slab-contiguous: [P, DT, 2, FT, 128] so one mt slab is one 8KB run
    w2x = np.ascontiguousarray(
        np.stack([w2hi, w2lo], axis=1).reshape(P, 2, FT, DT, P)
        .transpose(0, 3, 1, 2, 4))

    g2d = np.zeros((P, DT, P), f32)
    for mt in range(DT):
        np.fill_diagonal(g2d[:, mt, :],
                         WS_FFN1 * WS_FFN2 * g2[mt * P:(mt + 1) * P])

    def btile(bv_, ntile):
        return np.ascontiguousarray(bv_.reshape(ntile, P).T).astype(f32)

    E = np.zeros((16, DT, P), f32)
    for t in range(DT):
        for m in range(P):
            E[2 * t + m // HD, t, m] = 1.0

    return {
        "wqk": wqk, "wv": wtile(WS_V * Wvp, DT).astype(f8),
        "w1x": w1x, "w2x": w2x, "g2d": g2d.astype(bf),
        "bq": btile(bqp, DT), "bk": btile(bkp, DT),
        "bvb": np.ascontiguousarray(
            np.broadcast_to(WS_V * bvp, (P, D))).astype(bf),
        "b1": btile(WS_FFN1 * b1p, FT), "b2": btile(b2p, DT),
        "emat": E.astype(bf),
    }


def _per_core_inputs(inputs, shared):
    x = np.asarray(inputs["x"], np.float32)
    f8 = mybir.dt.np(F8)
    bf = ml_dtypes.bfloat16
    maps = []
    for c in range(NCORES):
        b, hf = c // 2, c % 2
        xTn = x[b].T.reshape(DT, P, S).transpose(1, 0, 2)
        if hf == 1:
            xTn = np.concatenate([xTn[:, :, SQ:], xTn[:, :, :SQ]], axis=2)
        xTn = np.ascontiguousarray(xTn)
        m = dict(shared)
        m["xbf"] = xTn.astype(bf)
        m["xf8"] = xTn.astype(f8)
        m["xsq8"] = (xTn * xTn).astype(f8)
        m["xh"] = np.ascontiguousarray(xTn[:, :, :SQ]).astype(bf)
        maps.append(m)
    return maps


def _get_sharded():
    """Build (once) the nc + jitted shard_map executable."""
    if "sharded" in _CACHE:
        return _CACHE["sharded"]

    import jax
    from jax.sharding import Mesh, PartitionSpec
    from jax.experimental.shard_map import shard_map
    from concourse import bass2jax
    from concourse import mybir as _mybir

    bass2jax.install_neuronx_cc_hook()
    nc = _build_nc()

    partition_name = (nc.partition_id_tensor.name
                      if nc.partition_id_tensor else None)
    in_names, out_names, out_avals, zero_shapes = [], [], [], []
    for alloc in nc.m.functions[0].allocations:
        if not isinstance(alloc, _mybir.MemoryLocationSet):
            continue
        name = alloc.memorylocations[0].name
        if alloc.kind == "ExternalInput":
            if name != partition_name:
                in_names.append(name)
        elif alloc.kind == "ExternalOutput":
            shape = tuple(alloc.tensor_shape)
            dtype = _mybir.dt.np(alloc.dtype)
            out_names.append(name)
            out_avals.append(jax.core.ShapedArray(shape, dtype))
            zero_shapes.append((shape, dtype))
    n_params = len(in_names)
    all_names = in_names + out_names
    if partition_name is not None:
        all_names = all_names + [partition_name]
    donate = tuple(range(n_params, n_params + len(out_names)))

    def _body(*args):
        operands = list(args)
        if partition_name is not None:
            operands.append(bass2jax.partition_id_tensor())
        outs = bass2jax._bass_exec_p.bind(
            *operands,
            out_avals=tuple(out_avals),
            in_names=tuple(all_names),
            out_names=tuple(out_names),
            lowering_input_output_aliases=(),
            sim_require_finite=True,
            sim_require_nnan=True,
            nc=nc,
        )
        return tuple(outs)

    devices = jax.devices()[:NCORES]
    mesh = Mesh(np.asarray(devices), ("core",))
    nin = n_params + len(out_names)
    sharded = jax.jit(
        shard_map(_body, mesh=mesh,
                  in_specs=(PartitionSpec("core"),) * nin,
                  out_specs=(PartitionSpec("core"),) * len(out_names),
                  check_rep=False),
        donate_argnums=donate, keep_unused=True)

    _CACHE["sharded"] = (nc, sharded, in_names, out_names, out_avals,
                         zero_shapes)
    return _CACHE["sharded"]


def _concat_inputs(in_maps):
    _, _, in_names, _, _, zero_shapes = _get_sharded()
    concat_in = [
        np.concatenate([np.asarray(in_maps[c][n]) for c in range(NCORES)],
                       axis=0)
        for n in in_names
    ]
    concat_zeros = [
        np.zeros((NCORES * s[0], *s[1:]), d) for (s, d) in zero_shapes
    ]
    return concat_in, concat_zeros


def _run(in_maps):
    nc, fn, in_names, out_names, out_avals, zero_shapes = _get_sharded()
    concat_in, concat_zeros = _concat_inputs(in_maps)
    outs = fn(*concat_in, *concat_zeros)
    res = []
    for c in range(NCORES):
        res.append({
            name: np.asarray(outs[i]).reshape(NCORES, *out_avals[i].shape)[c]
            for i, name in enumerate(out_names)
        })
    return res


def kernel(**inputs):
    shared = _prep_shared(inputs)
    in_maps = _per_core_inputs(inputs, shared)
    res = _run(in_maps)
    out = np.empty((B, S, D), np.float32)
    for c in range(NCORES):
        b, hf = c // 2, c % 2
        o = res[c]["OUT"]                       # [P, DT, SQ]
        out[b, hf * SQ:(hf + 1) * SQ, :] = o.transpose(2, 1, 0).reshape(SQ, D)
    return out


# revision 104
# speedup vs baseline: 1.1758x; 1.0534x over previous
"""Transformer block (pre-LN MHA + FFN) Trainium2 Bass kernel, fp8 edition.

Data-parallel over 8 cores: core c handles batch b=c//2, sequence half c%2.
Each core computes LN1 + K/V over the batch's FULL 2048 rows, Q/attention/
LN2/FFN over its own 1024 rows (rolled to columns 0:1024 host-side).

All heavy matmuls use fp8e4m3 DoubleRow perf mode (2 stacked 128-deep
contractions per instruction at 0.5 cycles/row):
 - QKV projections / FFN contract pairs of d-tiles of the fp8 activations.
 - Attention scores (64-deep per head) use a zero second stack on the Q
   side (step-sliced view onto a zeroed 9th tile) for 2x.
 - attn@V stacks pairs of k-row tiles of exp(scores) in fp8.
FFN weights are split hi+lo fp8 host-side (error compensation); the relu
activations optionally get the same split (BK_FFN2MM=3).  exp() is scaled
by 1/16 (bias -ln16) so unnormalized attn fits fp8.

LN rstd = exp(-0.5*ln(var+eps)) keeps the ACT engine on a single table
(natural_log_exp_and_others) so the softmax exp stream never reloads.

Emission interleaves FFN/LN2/normalize work for query-chunk pair p under
the ACT-bound attention windows of later chunks (background task queue).

Self-contained: hardcodes shapes B=4, S=2048, D=1024, H=16, FF=4096.
"""

import os

import numpy as np
import ml_dtypes

import concourse.bass as bass
import concourse.bacc as bacc
import concourse.tile as tile
from concourse import mybir

F32 = mybir.dt.float32
BF16 = mybir.dt.bfloat16
F8 = mybir.dt.float8e4
AF = mybir.ActivationFunctionType
OP = mybir.AluOpType
DRM = mybir.MatmulPerfMode.DoubleRow

B, S, D, H, FF = 4, 2048, 1024, 16, 4096
HD = D // H          # 64
P = 128
DT = D // P          # 8  d-tiles
FT = FF // P         # 32 ff-tiles
KT = S // P          # 16 k-row tiles
SQ = S // 2          # 1024 own q columns per core
AQ = 256             # attention q-chunk
NQC = SQ // AQ       # 4
EPS = 1e-5
EXPB = -5.545177444479562   # -ln(256): scale exp so fp8 numerator is safe
NCORES = 8

FFN1MM = int(os.environ.get("BK_FFN1MM", "3"))
FFN2MM = int(os.environ.get("BK_FFN2MM", "3"))
# Host-side weight scales keep fp8e4m3 (max 240, min normal 2^-6) in its
# normal range; descaled at evictions / the denominator stage.
WS_QKV = 32.0        # Wq/Wk scale (q,k descaled at evict)
WS_V = 2.0           # Wv scale (rides into attn numerator; denom stage x2)
WS_FFN1 = 32.0       # W1 scale (relu stage keeps 32x)
WS_FFN2 = 64.0       # W2 scale (final evict divides 32*64)

_CACHE = {}


def _build_nc():
    nc = bacc.Bacc("TRN2", target_bir_lowering=False, debug=False,
                   num_devices=NCORES)

    xbf = nc.dram_tensor("xbf", [P, DT, S], BF16, kind="ExternalInput")
    xf8 = nc.dram_tensor("xf8", [P, DT, S], F8, kind="ExternalInput")
    xsq8 = nc.dram_tensor("xsq8", [P, DT, S], F8, kind="ExternalInput")
    xh = nc.dram_tensor("xh", [P, DT, SQ], BF16, kind="ExternalInput")
    wqk = nc.dram_tensor("wqk", [P, 2, DT, D], F8, kind="ExternalInput")
    wv = nc.dram_tensor("wv", [P, DT, D], F8, kind="ExternalInput")
    w1x = nc.dram_tensor("w1x", [P, FT, 2, DT, P], F8, kind="ExternalInput")
    w2x = nc.dram_tensor("w2x", [P, DT, 2, FT, P], F8, kind="ExternalInput")
    g2d = nc.dram_tensor("g2d", [P, DT, P], BF16, kind="ExternalInput")
    bq = nc.dram_tensor("bq", [P, DT], F32, kind="ExternalInput")
    bk = nc.dram_tensor("bk", [P, DT], F32, kind="ExternalInput")

    b1 = nc.dram_tensor("b1", [P, FT], F32, kind="ExternalInput")
    b2 = nc.dram_tensor("b2", [P, DT], F32, kind="ExternalInput")
    emat = nc.dram_tensor("emat", [16, DT, P], BF16, kind="ExternalInput")
    OUT = nc.dram_tensor("OUT", [P, DT, SQ], F32, kind="ExternalOutput")

    repeat = int(os.environ.get("BASS_KERNEL_REPEAT", "1"))
    with tile.TileContext(nc) as tc:
        for _ in range(repeat):
            _emit(nc, tc, xbf, xf8, xsq8, xh, wqk, wv, w1x, w2x, g2d,
                  bq, bk, b1, b2, emat, OUT)
    nc.compile()
    return nc


def _emit(nc, tc, xbf_d, xf8_d, xsq8_d, xh_d, wqk_d, wv_d, w1x_d, w2x_d,
          g2d_d, bq_d, bk_d, b1_d, b2_d, emat_d, OUT_d):
    pools = {}
    pobj = {}

    def open_pool(name, bufs, space="SBUF"):
        cm = tc.tile_pool(name=name, bufs=bufs, space=space)
        pools[name] = cm
        pobj[name] = cm.__enter__()
        return pobj[name]

    def close_pool(name):
        pools.pop(name).__exit__(None, None, None)

    def dr(ps, lhsT, rhs, start, stop):
        nc.tensor.matmul(ps, lhsT, rhs, start=start, stop=stop,
                         perf_mode=DRM, skip_group_check=True)

    p_const = open_pool("consts", 1)
    p_ps = open_pool("psg", 2, space="PSUM")
    p_sc = open_pool("scps", 2, space="PSUM")
    p_aps = open_pool("apsps", 2, space="PSUM")

    # ---- constants ----
    ones8 = p_const.tile([P, 2, P], F8, tag="ones8")
    nc.vector.memset(ones8[:], 1.0)
    ones_bf = p_const.tile([P, P], BF16, tag="ones")
    nc.vector.memset(ones_bf[:], 1.0)
    eps_t = p_const.tile([P, 1], F32, tag="eps")
    nc.vector.memset(eps_t[:], EPS)
    expb_t = p_const.tile([P, 1], F32, tag="expb")
    nc.vector.memset(expb_t[:], EXPB)
    sb_bq = p_const.tile([P, DT], F32, tag="bq")
    nc.sync.dma_start(sb_bq[:], bq_d[:, :])
    sb_bk = p_const.tile([P, DT], F32, tag="bk")
    nc.sync.dma_start(sb_bk[:], bk_d[:, :])

    sb_b1 = p_const.tile([P, FT], F32, tag="b1")
    nc.sync.dma_start(sb_b1[:], b1_d[:, :])
    sb_b2 = p_const.tile([P, DT], F32, tag="b2")
    nc.sync.dma_start(sb_b2[:], b2_d[:, :])
    sb_emat = p_const.tile([16, DT, P], BF16, tag="emat")
    nc.sync.dma_start(sb_emat[:], emat_d[:, :, :])
    sb_g2d = p_const.tile([P, DT, P], BF16, tag="g2d")
    nc.sync.dma_start(sb_g2d[:], g2d_d[:, :, :])

    # ---- persistent activations ----
    p_fT = open_pool("fTp", 1)
    fT_bf = p_fT.tile([P, DT, SQ], BF16, tag="fTbf")
    fT8 = p_fT.tile([P, DT, SQ], F8, tag="fT8")
    fT8lo = (p_fT.tile([P, DT, SQ], F8, tag="fT8lo", name="fT8lo")
             if FFN1MM == 3 else None)

    p_qkv = open_pool("qkvout", 1)
    qT8 = p_qkv.tile([P, DT + 1, SQ], F8, tag="qT8")
    nc.vector.memset(qT8[:, DT, :], 0.0)
    kT8 = p_qkv.tile([P, DT + 1, S], F8, tag="kT8")
    nc.vector.memset(kT8[:, DT, :], 0.0)
    vaug = p_qkv.tile([P, KT, H, HD + 1], F8, tag="vaug")
    nc.vector.memset(vaug[:, :, :, HD:HD + 1], 1.0)

    p_hT = open_pool("hTp", 1)
    hT8 = p_hT.tile([P, DT, S], F8, tag="hT8")

    p_w = open_pool("wslab", 1)
    wv_s = p_w.tile([P, DT, D], F8, tag="wv_s")

    p_x = open_pool("xp", 1)
    xbf = p_x.tile([P, DT, S], BF16, tag="xbf")
    xf8 = p_x.tile([P, DT, S], F8, tag="xf8")
    xsq8 = p_x.tile([P, DT, S], F8, tag="xsq8")
    p_lt = open_pool("ln1tmp", 1)

    def ln_stats_smalls(ps1, ps2, tmp_pool, sfx, w):
        """psum sums -> (rstd bf16, nsb bf16) tiles of width w."""
        mu = tmp_pool.tile([P, w], F32, tag="mu" + sfx)
        nc.scalar.activation(mu[:], ps1, AF.Copy, bias=0.0, scale=1.0 / D)
        msq = tmp_pool.tile([P, w], F32, tag="msq" + sfx)
        nc.scalar.activation(msq[:], ps2, AF.Copy, bias=0.0, scale=1.0 / D)
        var = tmp_pool.tile([P, w], F32, tag="var" + sfx)
        nc.vector.tensor_mul(var[:], mu[:], mu[:])
        nc.vector.tensor_sub(var[:], msq[:], var[:])
        lnv = tmp_pool.tile([P, w], F32, tag="lnv" + sfx)
        nc.scalar.activation(lnv[:], var[:], AF.Ln, bias=eps_t[:], scale=1.0)
        rstd = tmp_pool.tile([P, w], BF16, tag="rstd" + sfx)
        with nc.allow_low_precision(reason="rstd bf16 feeds bf16 multiplies"):
            nc.scalar.activation(rstd[:], lnv[:], AF.Exp, bias=0.0, scale=-0.5)
        negmu = tmp_pool.tile([P, w], F32, tag="negmu" + sfx)
        nc.scalar.activation(negmu[:], mu[:], AF.Copy, bias=0.0, scale=-1.0)
        nsb = tmp_pool.tile([P, w], BF16, tag="nsb" + sfx)
        nc.vector.tensor_mul(nsb[:], negmu[:], rstd[:])
        return rstd, nsb

    # =========================================================
    # Phase A: LN1 (fp8 DR stats via host x/x^2) + V projection
    # =========================================================
    wqk_s = p_w.tile([P, 2, DT, D], F8, tag="wqk_s")

    for sc in range(4):
        ssl = bass.ts(sc, 512)
        nc.sync.dma_start(xbf[:, :, ssl], xbf_d[:, :, ssl])
        nc.sync.dma_start(xf8[:, :, ssl], xf8_d[:, :, ssl])
        nc.sync.dma_start(xsq8[:, :, ssl], xsq8_d[:, :, ssl])
        if sc == 0:
            # weights after the first LN1 chunk's inputs on the queue
            nc.sync.dma_start(wv_s[:], wv_d[:, :, :])
            nc.sync.dma_start(wqk_s[:], wqk_d[:, :, :, :])

        ps12 = p_sc.tile([P, 2, 512], F32, tag="scps")
        for i in range(4):
            dr(ps12[:, 0, :], ones8[:, :, :], xf8[:, 2 * i:2 * i + 2, ssl],
               start=(i == 0), stop=(i == 3))
        for i in range(4):
            dr(ps12[:, 1, :], ones8[:, :, :], xsq8[:, 2 * i:2 * i + 2, ssl],
               start=(i == 0), stop=(i == 3))
        rstd, nsb = ln_stats_smalls(ps12[:, 0, :], ps12[:, 1, :], p_lt,
                                    "a", 512)
        tmpb = p_lt.tile([P, DT, 512], BF16, tag="tmpb")
        nc.vector.tensor_tensor(
            tmpb[:], xbf[:, :, ssl],
            rstd[:, None, :].to_broadcast((P, DT, 512)), OP.mult)
        with nc.allow_low_precision(reason="hT fp8 matches matmul dtype"):
            nc.vector.tensor_tensor(
                hT8[:, 0:4, ssl], tmpb[:, 0:4, :],
                nsb[:, None, :].to_broadcast((P, 4, 512)), OP.add)
            nc.gpsimd.tensor_tensor(
                hT8[:, 4:DT, ssl], tmpb[:, 4:DT, :],
                nsb[:, None, :].to_broadcast((P, 4, 512)), OP.add)

        # V (Pool evict), K (ACT evict), Q (DVE evict) for this chunk
        for kt in range(4 * sc, 4 * sc + 4):
            for g in range(2):
                pv = p_ps.tile([P, 512], F32, tag="psg")
                for i in range(4):
                    dr(pv[:], hT8[:, 2 * i:2 * i + 2, bass.ts(kt, P)],
                       wv_s[:, 2 * i:2 * i + 2, bass.ts(g, 512)],
                       start=(i == 0), stop=(i == 3))
                # bv is folded into xh host-side (sum(probs) == 1)
                with nc.allow_low_precision(reason="v fp8 for fp8 attn"):
                    if g == 0:
                        nc.scalar.activation(
                            vaug[:, kt, 8 * g:8 * g + 8, 0:HD],
                            pv[:].rearrange("p (h d) -> p h d", d=HD),
                            AF.Copy, bias=0.0, scale=1.0)
                    else:
                        nc.vector.tensor_copy(
                            vaug[:, kt, 8 * g:8 * g + 8, 0:HD],
                            pv[:].rearrange("p (h d) -> p h d", d=HD))
        for t in range(DT):
            pk = p_ps.tile([P, 512], F32, tag="psg")
            for i in range(4):
                dr(pk[:], wqk_s[:, 1, 2 * i:2 * i + 2, bass.ts(t, P)],
                   hT8[:, 2 * i:2 * i + 2, ssl],
                   start=(i == 0), stop=(i == 3))
            with nc.allow_low_precision(reason="k fp8 for fp8 attn"):
                nc.scalar.activation(kT8[:, t, ssl], pk[:],
                                     AF.Identity, bias=sb_bk[:, t:t + 1],
                                     scale=1.0 / WS_QKV)
            if sc < 2:
                pq = p_ps.tile([P, 512], F32, tag="psg")
                for i in range(4):
                    dr(pq[:], wqk_s[:, 0, 2 * i:2 * i + 2, bass.ts(t, P)],
                       hT8[:, 2 * i:2 * i + 2, ssl],
                       start=(i == 0), stop=(i == 3))
                with nc.allow_low_precision(reason="q fp8 for fp8 attn"):
                    if t % 2 == 0:
                        nc.scalar.activation(qT8[:, t, ssl], pq[:],
                                             AF.Identity,
                                             bias=sb_bq[:, t:t + 1],
                                             scale=1.0 / WS_QKV)
                    else:
                        nc.vector.tensor_scalar(qT8[:, t, ssl], pq[:],
                                                1.0 / WS_QKV,
                                                sb_bq[:, t:t + 1],
                                                OP.mult, OP.add)

    close_pool("ln1tmp")
    close_pool("xp")
    close_pool("wslab")
    close_pool("hTp")

    # =========================================================
    # Phase C/D/E: attention chunks with interleaved background
    # normalize/LN2/FFN work for earlier chunks.
    # =========================================================
    p_att = open_pool("attn", 1)
    attn8 = p_att.tile([P, DT, SQ], F8, tag="attn8")
    p_ex = open_pool("expT", 2)
    p_st = open_pool("stage", 2)

    p_xh = open_pool("xhp", 1)
    xh = p_xh.tile([P, DT, SQ], BF16, tag="xh")
    for dt_ in range(DT):
        nc.sync.dma_start(xh[:, dt_, :], xh_d[:, dt_, :])
    rpad = p_xh.tile([16, SQ], BF16, tag="rpad")
    nc.vector.memset(rpad[:], 0.0)   # emat matmul reads all rows; keep finite

    p_yb = open_pool("ybp", 1)
    ybf = p_yb.tile([P, DT, SQ], BF16, tag="ybf")
    p_l2 = open_pool("ln2tmp", 1)

    p_fw = open_pool("ffnw", 2)
    p_fw1 = open_pool("ffnw1", 4)
    p_rl = open_pool("relu", 1)
    relu8 = p_rl.tile([P, FT, 512], F8, tag="relu8")
    relu8lo = (p_rl.tile([P, FT, 512], F8, tag="relu8lo", name="relu8lo")
               if FFN2MM == 3 else None)
    p_rbf = open_pool("relubf", 2)
    p_fo = open_pool("fout", 2)

    bg_hi = []   # normalize / LN2: tiny, unblock downstream
    bg_lo = []   # FFN1/FFN2 in dependency order
    pools_cur = {"ps": p_ps, "sc": p_sc}

    def bg_len():
        return len(bg_hi) + len(bg_lo)

    def drain(n):
        for _ in range(n):
            if bg_hi:
                bg_hi.pop(0)()
            elif bg_lo:
                bg_lo.pop(0)()
            else:
                return

    def attn_scores(t, i, qc):
        """Score burst + exp for head (t, i); returns the per-head ex tile."""
        pb = 64 * i
        qsl = bass.ts(qc, AQ)
        exh = p_ex.tile([P, 16, AQ], F8, tag="expT")
        for g in range(4):
            sc_ps = p_sc.tile([P, 4, AQ], F32, tag="scps")
            for kk in range(4):
                kt = 4 * g + kk
                nc.tensor.matmul(
                    sc_ps[:, kk, :],
                    kT8[pb:pb + 64, t:t + 2, bass.ts(kt, P)],
                    qT8[pb:pb + 64, t:DT + 1:DT - t, qsl],
                    start=(kk % 2 == 0), stop=(kk % 2 == 1),
                    perf_mode=DRM, skip_group_check=True)
            nc.scalar.activation(exh[:, 4 * g:4 * g + 4, :], sc_ps[:], AF.Exp,
                                 bias=expb_t[:], scale=0.125)
        return exh

    def attn_v(t, i, qc, exh):
        """attn@V + evict for head (t, i) using its exp tile."""
        h = 2 * t + i
        pb = 64 * i
        qsl = bass.ts(qc, AQ)
        aps = p_aps.tile([HD + 1, AQ], F32, tag="aps")
        for m in range(8):
            dr(aps[:, :], vaug[:, 2 * m:2 * m + 2, h, :],
               exh[:, 2 * m:2 * m + 2, :],
               start=(m == 0), stop=(m == 7))
        st = p_st.tile([HD, AQ], F8, tag="stage")
        with nc.allow_low_precision(reason="unnormalized attn fp8 (scaled)"):
            nc.vector.tensor_copy(st[:], aps[0:HD, :])
        std = p_st.tile([1, AQ], BF16, tag="staged")
        # denom * WS_V so rpad = 1/(WS_V * den) matches the v scale
        nc.vector.tensor_scalar(std[:], aps[HD:HD + 1, :], WS_V, None,
                                OP.mult)
        rp1 = p_st.tile([1, AQ], BF16, tag="stager")
        with nc.allow_low_precision(reason="softmax denom recip bf16"):
            nc.vector.reciprocal(rp1[:], std[:])
        nc.sync.dma_start(attn8[pb:pb + 64, t, qsl], st[:, :])
        nc.sync.dma_start(rpad[h:h + 1, qsl], rp1[:, :])
        if i == 1:
            bg_hi.append(mk_norm_task(qc, t))
            if t == DT - 1:
                bg_hi.extend(mk_ln2_tasks(qc))
                for ft in range(FT):
                    bg_lo.append(mk_ffn1_task(qc, ft))
                if qc % 2 == 1:
                    for mt in range(DT):
                        bg_lo.extend(mk_ffn2_tasks(qc // 2, mt))

    def mk_norm_task(qc, t):
        """Normalize + residual for d-tile t of chunk qc (heads 2t, 2t+1)."""
        def task():
            qsl = bass.ts(qc, AQ)
            rb = pools_cur["ps"].tile([P, AQ], F32, tag="psg", name="rb")
            nc.tensor.matmul(rb[:], sb_emat[:, t, :], rpad[:, qsl],
                             start=True, stop=True)
            t1 = p_l2.tile([P, AQ], F32, tag="t1")
            nc.vector.tensor_mul(t1[:], attn8[:, t, qsl], rb[:])
            nc.vector.tensor_add(ybf[:, t, qsl], t1[:], xh[:, t, qsl])
        return task

    def mk_ln2_tasks(qc):
        qsl = bass.ts(qc, AQ)
        st_ = {}

        def part_a():
            ysq = p_l2.tile([P, DT, AQ], BF16, tag="scr8a")
            nc.vector.tensor_mul(ysq[:], ybf[:, :, qsl], ybf[:, :, qsl])
            pool = pools_cur["sc"]
            ps12 = pool.tile([P, 2, AQ], F32,
                             tag=("scps" if pool is p_sc else "psg"),
                             name="ln2ps")
            for dt_ in range(DT):
                nc.tensor.matmul(ps12[:, 0, :], ones_bf[:], ybf[:, dt_, qsl],
                                 start=(dt_ == 0), stop=(dt_ == DT - 1))
            for dt_ in range(DT):
                nc.tensor.matmul(ps12[:, 1, :], ones_bf[:], ysq[:, dt_, :],
                                 start=(dt_ == 0), stop=(dt_ == DT - 1))
            st_["ps"] = ps12

        def part_b():
            ps12 = st_["ps"]
            rstd, nsb = ln_stats_smalls(ps12[:, 0, :], ps12[:, 1, :],
                                        p_l2, "b", AQ)
            tmpb = p_l2.tile([P, DT, AQ], BF16, tag="scr8a")
            nc.vector.tensor_tensor(
                tmpb[:], ybf[:, :, qsl],
                rstd[:, None, :].to_broadcast((P, DT, AQ)), OP.mult)
            nc.vector.tensor_tensor(
                fT_bf[:, :, qsl], tmpb[:],
                nsb[:, None, :].to_broadcast((P, DT, AQ)), OP.add)
            with nc.allow_low_precision(reason="f fp8 for fp8 FFN"):
                nc.gpsimd.tensor_tensor(
                    fT8[:, :, qsl], tmpb[:],
                    nsb[:, None, :].to_broadcast((P, DT, AQ)), OP.add)
                if fT8lo is not None:
                    nc.gpsimd.tensor_tensor(fT8lo[:, :, qsl],
                                            fT_bf[:, :, qsl],
                                            fT8[:, :, qsl], OP.subtract)
        return [part_a, part_b]

    def mk_ffn1_task(qc, ft):
        def task():
            qsl = bass.ts(qc, AQ)
            rsl = bass.ts(qc % 2, AQ)     # column range within the pair tile
            w1_s = p_fw1.tile([P, 2, DT, P], F8, tag="w1s")
            nc.sync.dma_start(w1_s[:], w1x_d[:, ft, :, :, :])
            pf = pools_cur["ps"].tile([P, AQ], F32, tag="psg", name="pf")
            mms = [(0, fT8), (1, fT8)]
            if FFN1MM == 3:
                mms.append((0, fT8lo))
            nmm = 0
            tot = 4 * len(mms)
            for hl, rhs in mms:
                for i in range(4):
                    dr(pf[:], w1_s[:, hl, 2 * i:2 * i + 2, :],
                       rhs[:, 2 * i:2 * i + 2, qsl],
                       start=(nmm == 0), stop=(nmm == tot - 1))
                    nmm += 1
            rbf = p_rbf.tile([P, AQ], BF16, tag="rbf")
            nc.vector.tensor_scalar(rbf[:], pf[:], sb_b1[:, ft:ft + 1],
                                    0.0, OP.add, OP.max)
            with nc.allow_low_precision(reason="relu fp8 for fp8 FFN2"):
                nc.vector.tensor_copy(relu8[:, ft, rsl], rbf[:])
                if relu8lo is not None:
                    nc.gpsimd.tensor_tensor(relu8lo[:, ft, rsl], rbf[:],
                                            relu8[:, ft, rsl], OP.subtract)
        return task

    def mk_ffn2_tasks(pr, mt):
        """FFN2 for output tile mt, split into ~1.5us micro-tasks."""
        psl = bass.ts(pr, 512)
        st_ = {}
        # (hl, rhs) matmul units: 32 hi/lo + 16 lo-relu, chunked by 12
        units = ([(0, relu8, j) for j in range(FT // 2)]
                 + [(1, relu8, j) for j in range(FT // 2)])
        if FFN2MM == 3:
            units += [(0, relu8lo, j) for j in range(FT // 2)]

        def c_first():
            w2_s = p_fw.tile([P, 2, FT, P], F8, tag="w2s")
            nc.scalar.dma_start(w2_s[:], w2x_d[:, mt, :, :, :])
            st_["w"] = w2_s
            st_["po"] = pools_cur["ps"].tile([P, 512], F32, tag="psg",
                                             name="po_f2")

        def mk_chunk(lo_i, hi_i, first):
            def chunk():
                if first:
                    c_first()
                w2_s, po = st_["w"], st_["po"]
                for u in range(lo_i, hi_i):
                    hl, rhs, j = units[u]
                    dr(po[:], w2_s[:, hl, 2 * j:2 * j + 2, :],
                       rhs[:, 2 * j:2 * j + 2, :],
                       start=(u == 0), stop=False)
            return chunk

        def c_last():
            w2_s, po = st_["w"], st_["po"]
            nc.tensor.matmul(po[:], sb_g2d[:, mt, :], fT_bf[:, mt, psl],
                             start=False, stop=True, skip_group_check=True)
            ot = p_fo.tile([P, 512], F32, tag="ot")
            nc.vector.tensor_scalar(ot[:], po[:], 1.0 / (WS_FFN1 * WS_FFN2),
                                    sb_b2[:, mt:mt + 1], OP.mult, OP.add)
            nc.scalar.dma_start(OUT_d[:, mt, psl], ot[:])

        n = len(units)
        step = 12
        tasks = []
        for s in range(0, n, step):
            tasks.append(mk_chunk(s, min(s + step, n), s == 0))
        tasks.append(c_last)
        return tasks

    prev = None
    for qc in range(NQC):
        for t in range(DT):
            for i in range(2):
                exh = attn_scores(t, i, qc)
                if prev is not None:
                    attn_v(*prev)
                prev = (t, i, qc, exh)
                drain(4)
    attn_v(*prev)
    prev = None

    # attention psum pools are done; hand their banks to the FFN tail
    close_pool("apsps")
    close_pool("scps")
    p_pst = open_pool("tailps", 4, space="PSUM")
    pools_cur["ps"] = p_pst
    pools_cur["sc"] = p_pst
    drain(bg_len())

    close_pool("fout")
    close_pool("relubf")
    close_pool("relu")
    close_pool("ffnw1")
    close_pool("ffnw")
    close_pool("ln2tmp")
    close_pool("ybp")
    close_pool("xhp")
    close_pool("stage")
    close_pool("expT")
    close_pool("attn")
    close_pool("qkvout")
    close_pool("fTp")
    close_pool("tailps")
    close_pool("psg")
    close_pool("consts")


def _prep_shared(inputs):
    """Host-side weight preprocessing (shared across cores)."""
    f32 = np.float32
    g1 = np.asarray(inputs["g1"], f32)
    beta1 = np.asarray(inputs["beta1"], f32)
    g2 = np.asarray(inputs["g2"], f32)
    beta2 = np.asarray(inputs["beta2"], f32)
    Wq = np.asarray(inputs["Wq"], f32)
    Wk = np.asarray(inputs["Wk"], f32)
    Wv = np.asarray(inputs["Wv"], f32)
    W1 = np.asarray(inputs["W1"], f32)
    W2 = np.asarray(inputs["W2"], f32)

    def fold(Wm, bm):
        Wp = Wm * g1[:, None]
        bp = np.asarray(inputs[bm], f32) + beta1 @ Wm
        return Wp, bp

    Wqp, bqp = fold(Wq, "bq")
    Wkp, bkp = fold(Wk, "bk")
    Wvp, bvp = fold(Wv, "bv")
    W1p = W1 * g2[:, None]
    b1p = np.asarray(inputs["b1"], f32) + beta2 @ W1
    b2p = np.asarray(inputs["b2"], f32) + beta2

    f8 = mybir.dt.np(F8)
    bf = ml_dtypes.bfloat16

    def wtile(Wm, ntile):
        # [K, N] -> [P, ntile, N] with K = ntile*P (partition-major k)
        return np.ascontiguousarray(
            Wm.reshape(ntile, P, Wm.shape[1]).transpose(1, 0, 2))

    def hilo(Wt):
        hi = Wt.astype(f8)
        lo = (Wt - hi.astype(f32)).astype(f8)
        return hi, lo

    wq_t = wtile(WS_QKV * Wqp, DT).astype(f8)
    wk_t = wtile(WS_QKV * Wkp, DT).astype(f8)
    wqk = np.ascontiguousarray(np.stack([wq_t, wk_t], axis=1))
    w1hi, w1lo = hilo(wtile(WS_FFN1 * W1p, DT))
    # slab-contiguous: [P, FT, 2, DT, 128] so one ft slab is one 2KB run
    w1x = np.ascontiguousarray(
        np.stack([w1hi, w1lo], axis=1).reshape(P, 2, DT, FT, P)
        .transpose(0, 3, 1, 2, 4))
    w2hi, w2lo = hilo(wtile(WS_FFN2 * W2, FT))
    # slab-contiguous: [P, DT, 2, FT, 128] so one mt slab is one 8KB run
    w2x = np.ascontiguousarray(
        np.stack([w2hi, w2lo], axis=1).reshape(P, 2, FT, DT, P)
        .transpose(0, 3, 1, 2, 4))

    g2d = np.zeros((P, DT, P), f32)
    for mt in range(DT):
        np.fill_diagonal(g2d[:, mt, :],
                         WS_FFN1 * WS_FFN2 * g2[mt * P:(mt + 1) * P])

    def btile(bv_, ntile):
        return np.ascontiguousarray(bv_.reshape(ntile, P).T).astype(f32)

    E = np.zeros((16, DT, P), f32)
    for t in range(DT):
        for m in range(P):
            E[2 * t + m // HD, t, m] = 1.0

    return {
        "wqk": wqk, "wv": wtile(WS_V * Wvp, DT).astype(f8),
        "w1x": w1x, "w2x": w2x, "g2d": g2d.astype(bf),
        "bq": btile(bqp, DT), "bk": btile(bkp, DT),
        "_bv_fold": btile(bvp, DT),
        "b1": btile(WS_FFN1 * b1p, FT), "b2": btile(b2p, DT),
        "emat": E.astype(bf),
    }


def _per_core_inputs(inputs, shared):
    x = np.asarray(inputs["x"], np.float32)
    f8 = mybir.dt.np(F8)
    bf = ml_dtypes.bfloat16
    maps = []
    for c in range(NCORES):
        b, hf = c // 2, c % 2
        xTn = x[b].T.reshape(DT, P, S).transpose(1, 0, 2)
        if hf == 1:
            xTn = np.concatenate([xTn[:, :, SQ:], xTn[:, :, :SQ]], axis=2)
        xTn = np.ascontiguousarray(xTn)
        m = dict(shared)
        m["xbf"] = xTn.astype(bf)
        m["xf8"] = xTn.astype(f8)
        m["xsq8"] = (xTn * xTn).astype(f8)
        # bv folded into the attention residual (sum of probs == 1)
        m["xh"] = np.ascontiguousarray(
            xTn[:, :, :SQ] + shared["_bv_fold"][:, :, None]).astype(bf)
        maps.append(m)
    return maps


def _get_sharded():
    """Build (once) the nc + jitted shard_map executable."""
    if "sharded" in _CACHE:
        return _CACHE["sharded"]

    import jax
    from jax.sharding import Mesh, PartitionSpec
    from jax.experimental.shard_map import shard_map
    from concourse import bass2jax
    from concourse import mybir as _mybir

    bass2jax.install_neuronx_cc_hook()
    nc = _build_nc()

    partition_name = (nc.partition_id_tensor.name
                      if nc.partition_id_tensor else None)
    in_names, out_names, out_avals, zero_shapes = [], [], [], []
    for alloc in nc.m.functions[0].allocations:
        if not isinstance(alloc, _mybir.MemoryLocationSet):
            continue
        name = alloc.memorylocations[0].name
        if alloc.kind == "ExternalInput":
            if name != partition_name:
                in_names.append(name)
        elif alloc.kind == "ExternalOutput":
            shape = tuple(alloc.tensor_shape)
            dtype = _mybir.dt.np(alloc.dtype)
            out_names.append(name)
            out_avals.append(jax.core.ShapedArray(shape, dtype))
            zero_shapes.append((shape, dtype))
    n_params = len(in_names)
    all_names = in_names + out_names
    if partition_name is not None:
        all_names = all_names + [partition_name]
    donate = tuple(range(n_params, n_params + len(out_names)))

    def _body(*args):
        operands = list(args)
        if partition_name is not None:
            operands.append(bass2jax.partition_id_tensor())
        outs = bass2jax._bass_exec_p.bind(
            *operands,
            out_avals=tuple(out_avals),
            in_names=tuple(all_names),
            out_names=tuple(out_names),
            lowering_input_output_aliases=(),
            sim_require_finite=True,
            sim_require_nnan=True,
            nc=nc,
        )
        return tuple(outs)

    devices = jax.devices()[:NCORES]
    mesh = Mesh(np.asarray(devices), ("core",))
    nin = n_params + len(out_names)
    sharded = jax.jit(
        shard_map(_body, mesh=mesh,
                  in_specs=(PartitionSpec("core"),) * nin,
                  out_specs=(PartitionSpec("core"),) * len(out_names),
                  check_rep=False),
        donate_argnums=donate, keep_unused=True)

    _CACHE["sharded"] = (nc, sharded, in_names, out_names, out_avals,
                         zero_shapes)
    return _CACHE["sharded"]


def _concat_inputs(in_maps):
    _, _, in_names, _, _, zero_shapes = _get_sharded()
    concat_in = [
        np.concatenate([np.asarray(in_maps[c][n]) for c in range(NCORES)],
                       axis=0)
        for n in in_names
    ]
    concat_zeros = [
        np.zeros((NCORES * s[0], *s[1:]), d) for (s, d) in zero_shapes
    ]
    return concat_in, concat_zeros


def _run(in_maps):
    nc, fn, in_names, out_names, out_avals, zero_shapes = _get_sharded()
    concat_in, concat_zeros = _concat_inputs(in_maps)
    outs = fn(*concat_in, *concat_zeros)
    res = []
    for c in range(NCORES):
        res.append({
            name: np.asarray(outs[i]).reshape(NCORES, *out_avals[i].shape)[c]
            for i, name in enumerate(out_names)
        })
    return res


def kernel(**inputs):
    shared = _prep_shared(inputs)
    in_maps = _per_core_inputs(inputs, shared)
    res = _run(in_maps)
    out = np.empty((B, S, D), np.float32)
    for c in range(NCORES):
        b, hf = c // 2, c % 2
        o = res[c]["OUT"]                       # [P, DT, SQ]
        out[b, hf * SQ:(hf + 1) * SQ, :] = o.transpose(2, 1, 0).reshape(SQ, D)
    return out


# revision 108
# speedup vs baseline: 1.2184x; 1.0362x over previous
"""Transformer block (pre-LN MHA + FFN) Trainium2 Bass kernel, fp8 edition.

Data-parallel over 8 cores: core c handles batch b=c//2, sequence half c%2.
Each core computes LN1 + K/V over the batch's FULL 2048 rows, Q/attention/
LN2/FFN over its own 1024 rows (rolled to columns 0:1024 host-side).

All heavy matmuls use fp8e4m3 DoubleRow perf mode (2 stacked 128-deep
contractions per instruction at 0.5 cycles/row):
 - QKV projections / FFN contract pairs of d-tiles of the fp8 activations.
 - Attention scores (64-deep per head) use a zero second stack on the Q
   side (step-sliced view onto a zeroed 9th tile) for 2x.
 - attn@V stacks pairs of k-row tiles of exp(scores) in fp8.
FFN weights are split hi+lo fp8 host-side (error compensation); the relu
activations optionally get the same split (BK_FFN2MM=3).  exp() is scaled
by 1/16 (bias -ln16) so unnormalized attn fits fp8.

LN rstd = exp(-0.5*ln(var+eps)) keeps the ACT engine on a single table
(natural_log_exp_and_others) so the softmax exp stream never reloads.

Emission interleaves FFN/LN2/normalize work for query-chunk pair p under
the ACT-bound attention windows of later chunks (background task queue).

Self-contained: hardcodes shapes B=4, S=2048, D=1024, H=16, FF=4096.
"""

import os

import numpy as np
import ml_dtypes

import concourse.bass as bass
import concourse.bacc as bacc
import concourse.tile as tile
from concourse import mybir

F32 = mybir.dt.float32
BF16 = mybir.dt.bfloat16
F8 = mybir.dt.float8e4
AF = mybir.ActivationFunctionType
OP = mybir.AluOpType
DRM = mybir.MatmulPerfMode.DoubleRow

B, S, D, H, FF = 4, 2048, 1024, 16, 4096
HD = D // H          # 64
P = 128
DT = D // P          # 8  d-tiles
FT = FF // P         # 32 ff-tiles
KT = S // P          # 16 k-row tiles
SQ = S // 2          # 1024 own q columns per core
AQ = 256             # attention q-chunk
NQC = SQ // AQ       # 4
EPS = 1e-5
EXPB = -5.545177444479562   # -ln(256): scale exp so fp8 numerator is safe
NCORES = 8

FFN1MM = int(os.environ.get("BK_FFN1MM", "3"))
FFN2MM = int(os.environ.get("BK_FFN2MM", "3"))
# Host-side weight scales keep fp8e4m3 (max 240, min normal 2^-6) in its
# normal range; descaled at evictions / the denominator stage.
WS_QKV = 32.0        # Wq/Wk scale (q,k descaled at evict)
WS_V = 2.0           # Wv scale (rides into attn numerator; denom stage x2)
WS_FFN1 = 32.0       # W1 scale (relu stage keeps 32x)
WS_FFN2 = 64.0       # W2 scale (final evict divides 32*64)

_CACHE = {}


def _build_nc():
    nc = bacc.Bacc("TRN2", target_bir_lowering=False, debug=False,
                   num_devices=NCORES)

    xbf = nc.dram_tensor("xbf", [P, DT, S], BF16, kind="ExternalInput")
    xf8 = nc.dram_tensor("xf8", [P, DT, S], F8, kind="ExternalInput")
    xsq8 = nc.dram_tensor("xsq8", [P, DT, S], F8, kind="ExternalInput")
    xh = nc.dram_tensor("xh", [P, DT, SQ], BF16, kind="ExternalInput")
    wqk = nc.dram_tensor("wqk", [P, 2, DT, D], F8, kind="ExternalInput")
    wv = nc.dram_tensor("wv", [P, DT, D], F8, kind="ExternalInput")
    w1x = nc.dram_tensor("w1x", [P, FT, 2, DT, P], F8, kind="ExternalInput")
    w2x = nc.dram_tensor("w2x", [P, DT, 2, FT, P], F8, kind="ExternalInput")
    g2d = nc.dram_tensor("g2d", [P, DT, P], BF16, kind="ExternalInput")
    bq = nc.dram_tensor("bq", [P, DT], F32, kind="ExternalInput")
    bk = nc.dram_tensor("bk", [P, DT], F32, kind="ExternalInput")

    b1 = nc.dram_tensor("b1", [P, FT], F32, kind="ExternalInput")
    b2 = nc.dram_tensor("b2", [P, DT], F32, kind="ExternalInput")
    emat = nc.dram_tensor("emat", [16, DT, P], BF16, kind="ExternalInput")
    OUT = nc.dram_tensor("OUT", [P, DT, SQ], F32, kind="ExternalOutput")

    repeat = int(os.environ.get("BASS_KERNEL_REPEAT", "1"))
    with tile.TileContext(nc) as tc:
        for _ in range(repeat):
            _emit(nc, tc, xbf, xf8, xsq8, xh, wqk, wv, w1x, w2x, g2d,
                  bq, bk, b1, b2, emat, OUT)
    nc.compile()
    return nc


def _emit(nc, tc, xbf_d, xf8_d, xsq8_d, xh_d, wqk_d, wv_d, w1x_d, w2x_d,
          g2d_d, bq_d, bk_d, b1_d, b2_d, emat_d, OUT_d):
    pools = {}
    pobj = {}

    def open_pool(name, bufs, space="SBUF"):
        cm = tc.tile_pool(name=name, bufs=bufs, space=space)
        pools[name] = cm
        pobj[name] = cm.__enter__()
        return pobj[name]

    def close_pool(name):
        pools.pop(name).__exit__(None, None, None)

    def dr(ps, lhsT, rhs, start, stop):
        nc.tensor.matmul(ps, lhsT, rhs, start=start, stop=stop,
                         perf_mode=DRM, skip_group_check=True)

    p_const = open_pool("consts", 1)
    p_ps = open_pool("psg", 2, space="PSUM")
    p_sc = open_pool("scps", 2, space="PSUM")
    p_aps = open_pool("apsps", 2, space="PSUM")

    # ---- constants ----
    ones8 = p_const.tile([P, 2, P], F8, tag="ones8")
    nc.vector.memset(ones8[:], 1.0)
    ones_bf = p_const.tile([P, P], BF16, tag="ones")
    nc.vector.memset(ones_bf[:], 1.0)
    eps_t = p_const.tile([P, 1], F32, tag="eps")
    nc.vector.memset(eps_t[:], EPS)
    expb_t = p_const.tile([P, 1], F32, tag="expb")
    nc.vector.memset(expb_t[:], EXPB)
    sb_bq = p_const.tile([P, DT], F32, tag="bq")
    nc.sync.dma_start(sb_bq[:], bq_d[:, :])
    sb_bk = p_const.tile([P, DT], F32, tag="bk")
    nc.sync.dma_start(sb_bk[:], bk_d[:, :])

    sb_b1 = p_const.tile([P, FT], F32, tag="b1")
    nc.sync.dma_start(sb_b1[:], b1_d[:, :])
    sb_b2 = p_const.tile([P, DT], F32, tag="b2")
    nc.sync.dma_start(sb_b2[:], b2_d[:, :])
    sb_emat = p_const.tile([16, DT, P], BF16, tag="emat")
    nc.sync.dma_start(sb_emat[:], emat_d[:, :, :])
    sb_g2d = p_const.tile([P, DT, P], BF16, tag="g2d")
    nc.sync.dma_start(sb_g2d[:], g2d_d[:, :, :])

    # ---- persistent activations ----
    p_fT = open_pool("fTp", 1)
    fT_bf = p_fT.tile([P, DT, SQ], BF16, tag="fTbf")
    fT8 = p_fT.tile([P, DT, SQ], F8, tag="fT8")
    fT8lo = (p_fT.tile([P, DT, SQ], F8, tag="fT8lo", name="fT8lo")
             if FFN1MM == 3 else None)

    p_qkv = open_pool("qkvout", 1)
    qT8 = p_qkv.tile([P, DT + 1, SQ], F8, tag="qT8")
    nc.vector.memset(qT8[:, DT, :], 0.0)
    kT8 = p_qkv.tile([P, DT + 1, S], F8, tag="kT8")
    nc.vector.memset(kT8[:, DT, :], 0.0)
    vaug = p_qkv.tile([P, KT, H, HD + 1], F8, tag="vaug")
    nc.vector.memset(vaug[:, :, :, HD:HD + 1], 1.0)

    p_hT = open_pool("hTp", 1)
    hT8 = p_hT.tile([P, DT, S], F8, tag="hT8")

    p_w = open_pool("wslab", 1)
    wv_s = p_w.tile([P, DT, D], F8, tag="wv_s")

    p_x = open_pool("xp", 1)
    xbf = p_x.tile([P, DT, S], BF16, tag="xbf")
    xf8 = p_x.tile([P, DT, S], F8, tag="xf8")
    xsq8 = p_x.tile([P, DT, S], F8, tag="xsq8")
    p_lt = open_pool("ln1tmp", 1)

    def ln_stats_smalls(ps1, ps2, tmp_pool, sfx, w, act_smalls=True):
        """psum sums -> (rstd bf16, nsb bf16) tiles of width w."""
        mu = tmp_pool.tile([P, w], F32, tag="mu" + sfx)
        msq = tmp_pool.tile([P, w], F32, tag="msq" + sfx)
        if act_smalls:
            nc.scalar.activation(mu[:], ps1, AF.Copy, bias=0.0, scale=1.0 / D)
            nc.scalar.activation(msq[:], ps2, AF.Copy, bias=0.0,
                                 scale=1.0 / D)
        else:
            nc.vector.tensor_scalar(mu[:], ps1, 1.0 / D, None, OP.mult)
            nc.vector.tensor_scalar(msq[:], ps2, 1.0 / D, None, OP.mult)
        var = tmp_pool.tile([P, w], F32, tag="var" + sfx)
        nc.vector.tensor_mul(var[:], mu[:], mu[:])
        nc.vector.tensor_sub(var[:], msq[:], var[:])
        lnv = tmp_pool.tile([P, w], F32, tag="lnv" + sfx)
        nc.scalar.activation(lnv[:], var[:], AF.Ln, bias=eps_t[:], scale=1.0)
        rstd = tmp_pool.tile([P, w], BF16, tag="rstd" + sfx)
        with nc.allow_low_precision(reason="rstd bf16 feeds bf16 multiplies"):
            nc.scalar.activation(rstd[:], lnv[:], AF.Exp, bias=0.0, scale=-0.5)
        nsb = tmp_pool.tile([P, w], BF16, tag="nsb" + sfx)
        negmu = tmp_pool.tile([P, w], F32, tag="negmu" + sfx)
        if act_smalls:
            nc.scalar.activation(negmu[:], mu[:], AF.Copy, bias=0.0,
                                 scale=-1.0)
        else:
            nc.vector.tensor_scalar(negmu[:], mu[:], -1.0, None, OP.mult)
        nc.vector.tensor_mul(nsb[:], negmu[:], rstd[:])
        return rstd, nsb

    # =========================================================
    # Phase A: LN1 (fp8 DR stats via host x/x^2) + V projection
    # =========================================================
    wqk_s = p_w.tile([P, 2, DT, D], F8, tag="wqk_s")

    for sc in range(4):
        ssl = bass.ts(sc, 512)
        nc.sync.dma_start(xbf[:, :, ssl], xbf_d[:, :, ssl])
        nc.sync.dma_start(xf8[:, :, ssl], xf8_d[:, :, ssl])
        nc.sync.dma_start(xsq8[:, :, ssl], xsq8_d[:, :, ssl])
        if sc == 0:
            # weights after the first LN1 chunk's inputs on the queue
            nc.sync.dma_start(wv_s[:], wv_d[:, :, :])
            nc.sync.dma_start(wqk_s[:], wqk_d[:, :, :, :])

        ps12 = p_sc.tile([P, 2, 512], F32, tag="scps")
        for i in range(4):
            dr(ps12[:, 0, :], ones8[:, :, :], xf8[:, 2 * i:2 * i + 2, ssl],
               start=(i == 0), stop=(i == 3))
        for i in range(4):
            dr(ps12[:, 1, :], ones8[:, :, :], xsq8[:, 2 * i:2 * i + 2, ssl],
               start=(i == 0), stop=(i == 3))
        rstd, nsb = ln_stats_smalls(ps12[:, 0, :], ps12[:, 1, :], p_lt,
                                    "a", 512)
        tmpb = p_lt.tile([P, DT, 512], BF16, tag="tmpb")
        nc.vector.tensor_tensor(
            tmpb[:], xbf[:, :, ssl],
            rstd[:, None, :].to_broadcast((P, DT, 512)), OP.mult)
        with nc.allow_low_precision(reason="hT fp8 matches matmul dtype"):
            nc.vector.tensor_tensor(
                hT8[:, 0:4, ssl], tmpb[:, 0:4, :],
                nsb[:, None, :].to_broadcast((P, 4, 512)), OP.add)
            nc.gpsimd.tensor_tensor(
                hT8[:, 4:DT, ssl], tmpb[:, 4:DT, :],
                nsb[:, None, :].to_broadcast((P, 4, 512)), OP.add)

        # V (Pool evict), K (ACT evict), Q (DVE evict) for this chunk
        for kt in range(4 * sc, 4 * sc + 4):
            for g in range(2):
                pv = p_ps.tile([P, 512], F32, tag="psg")
                for i in range(4):
                    dr(pv[:], hT8[:, 2 * i:2 * i + 2, bass.ts(kt, P)],
                       wv_s[:, 2 * i:2 * i + 2, bass.ts(g, 512)],
                       start=(i == 0), stop=(i == 3))
                # bv is folded into xh host-side (sum(probs) == 1)
                with nc.allow_low_precision(reason="v fp8 for fp8 attn"):
                    if g == 0:
                        nc.scalar.activation(
                            vaug[:, kt, 8 * g:8 * g + 8, 0:HD],
                            pv[:].rearrange("p (h d) -> p h d", d=HD),
                            AF.Copy, bias=0.0, scale=1.0)
                    else:
                        nc.vector.tensor_copy(
                            vaug[:, kt, 8 * g:8 * g + 8, 0:HD],
                            pv[:].rearrange("p (h d) -> p h d", d=HD))
        for t in range(DT):
            pk = p_ps.tile([P, 512], F32, tag="psg")
            for i in range(4):
                dr(pk[:], wqk_s[:, 1, 2 * i:2 * i + 2, bass.ts(t, P)],
                   hT8[:, 2 * i:2 * i + 2, ssl],
                   start=(i == 0), stop=(i == 3))
            with nc.allow_low_precision(reason="k fp8 for fp8 attn"):
                nc.scalar.activation(kT8[:, t, ssl], pk[:],
                                     AF.Identity, bias=sb_bk[:, t:t + 1],
                                     scale=1.0 / WS_QKV)
            if sc < 2:
                pq = p_ps.tile([P, 512], F32, tag="psg")
                for i in range(4):
                    dr(pq[:], wqk_s[:, 0, 2 * i:2 * i + 2, bass.ts(t, P)],
                       hT8[:, 2 * i:2 * i + 2, ssl],
                       start=(i == 0), stop=(i == 3))
                with nc.allow_low_precision(reason="q fp8 for fp8 attn"):
                    if t % 2 == 0:
                        nc.scalar.activation(qT8[:, t, ssl], pq[:],
                                             AF.Identity,
                                             bias=sb_bq[:, t:t + 1],
                                             scale=1.0 / WS_QKV)
                    else:
                        nc.vector.tensor_scalar(qT8[:, t, ssl], pq[:],
                                                1.0 / WS_QKV,
                                                sb_bq[:, t:t + 1],
                                                OP.mult, OP.add)

    close_pool("ln1tmp")
    close_pool("xp")
    close_pool("wslab")
    close_pool("hTp")

    # =========================================================
    # Phase C/D/E: attention chunks with interleaved background
    # normalize/LN2/FFN work for earlier chunks.
    # =========================================================
    p_att = open_pool("attn", 1)
    attn8 = p_att.tile([P, DT, SQ], F8, tag="attn8")
    p_ex = open_pool("expT", 2)
    p_st = open_pool("stage", 2)

    p_xh = open_pool("xhp", 1)
    xh = p_xh.tile([P, DT, SQ], BF16, tag="xh")
    for dt_ in range(DT):
        nc.sync.dma_start(xh[:, dt_, :], xh_d[:, dt_, :])
    rpad = p_xh.tile([16, SQ], BF16, tag="rpad")
    nc.vector.memset(rpad[:], 0.0)   # emat matmul reads all rows; keep finite

    p_yb = open_pool("ybp", 1)
    ybf = p_yb.tile([P, DT, SQ], BF16, tag="ybf")
    p_l2 = open_pool("ln2tmp", 1)

    p_fw = open_pool("ffnw", 2)
    p_fw1 = open_pool("ffnw1", 4)
    p_rl = open_pool("relu", 1)
    relu8 = p_rl.tile([P, FT, 512], F8, tag="relu8")
    relu8lo = (p_rl.tile([P, FT, 512], F8, tag="relu8lo", name="relu8lo")
               if FFN2MM == 3 else None)
    p_rbf = open_pool("relubf", 2)
    p_fo = open_pool("fout", 2)

    bg_hi = []   # normalize / LN2: tiny, unblock downstream
    bg_lo = []   # FFN1/FFN2 in dependency order
    pools_cur = {"ps": p_ps, "sc": p_sc}

    def bg_len():
        return len(bg_hi) + len(bg_lo)

    def drain(n):
        for _ in range(n):
            if bg_hi:
                bg_hi.pop(0)()
            elif bg_lo:
                bg_lo.pop(0)()
            else:
                return

    def attn_scores(t, i, qc):
        """Score burst + exp for head (t, i); returns the per-head ex tile."""
        pb = 64 * i
        qsl = bass.ts(qc, AQ)
        exh = p_ex.tile([P, 16, AQ], F8, tag="expT")
        for g in range(4):
            sc_ps = p_sc.tile([P, 4, AQ], F32, tag="scps")
            for kk in range(4):
                kt = 4 * g + kk
                nc.tensor.matmul(
                    sc_ps[:, kk, :],
                    kT8[pb:pb + 64, t:t + 2, bass.ts(kt, P)],
                    qT8[pb:pb + 64, t:DT + 1:DT - t, qsl],
                    start=(kk % 2 == 0), stop=(kk % 2 == 1),
                    perf_mode=DRM, skip_group_check=True)
            nc.scalar.activation(exh[:, 4 * g:4 * g + 4, :], sc_ps[:], AF.Exp,
                                 bias=expb_t[:], scale=0.125)
        return exh

    def attn_v(t, i, qc, exh):
        """attn@V + evict for head (t, i) using its exp tile."""
        h = 2 * t + i
        pb = 64 * i
        qsl = bass.ts(qc, AQ)
        aps = p_aps.tile([HD + 1, AQ], F32, tag="aps")
        for m in range(8):
            dr(aps[:, :], vaug[:, 2 * m:2 * m + 2, h, :],
               exh[:, 2 * m:2 * m + 2, :],
               start=(m == 0), stop=(m == 7))
        st = p_st.tile([HD, AQ], F8, tag="stage")
        with nc.allow_low_precision(reason="unnormalized attn fp8 (scaled)"):
            nc.vector.tensor_copy(st[:], aps[0:HD, :])
        std = p_st.tile([1, AQ], BF16, tag="staged")
        # denom * WS_V so rpad = 1/(WS_V * den) matches the v scale
        nc.vector.tensor_scalar(std[:], aps[HD:HD + 1, :], WS_V, None,
                                OP.mult)
        rp1 = p_st.tile([1, AQ], BF16, tag="stager")
        with nc.allow_low_precision(reason="softmax denom recip bf16"):
            nc.vector.reciprocal(rp1[:], std[:])
        nc.sync.dma_start(attn8[pb:pb + 64, t, qsl], st[:, :])
        nc.sync.dma_start(rpad[h:h + 1, qsl], rp1[:, :])
        if i == 1:
            bg_hi.append(mk_norm_task(qc, t))
            if t == DT - 1:
                bg_hi.extend(mk_ln2_tasks(qc))
                for ft in range(FT):
                    bg_lo.append(mk_ffn1_task(qc, ft))
                if qc % 2 == 1:
                    for mt in range(DT):
                        bg_lo.extend(mk_ffn2_tasks(qc // 2, mt))

    def mk_norm_task(qc, t):
        """Normalize + residual for d-tile t of chunk qc (heads 2t, 2t+1)."""
        def task():
            qsl = bass.ts(qc, AQ)
            rb = pools_cur["ps"].tile([P, AQ], F32, tag="psg", name="rb")
            nc.tensor.matmul(rb[:], sb_emat[:, t, :], rpad[:, qsl],
                             start=True, stop=True)
            t1 = p_l2.tile([P, AQ], F32, tag="t1")
            nc.vector.tensor_mul(t1[:], attn8[:, t, qsl], rb[:])
            nc.vector.tensor_add(ybf[:, t, qsl], t1[:], xh[:, t, qsl])
        return task

    def mk_ln2_tasks(qc):
        qsl = bass.ts(qc, AQ)
        st_ = {}

        def part_a():
            ysq = p_l2.tile([P, DT, AQ], BF16, tag="scr8a")
            nc.vector.tensor_mul(ysq[:], ybf[:, :, qsl], ybf[:, :, qsl])
            pool = pools_cur["sc"]
            ps12 = pool.tile([P, 2, AQ], F32,
                             tag=("scps" if pool is p_sc else "psg"),
                             name="ln2ps")
            for dt_ in range(DT):
                nc.tensor.matmul(ps12[:, 0, :], ones_bf[:], ybf[:, dt_, qsl],
                                 start=(dt_ == 0), stop=(dt_ == DT - 1))
            for dt_ in range(DT):
                nc.tensor.matmul(ps12[:, 1, :], ones_bf[:], ysq[:, dt_, :],
                                 start=(dt_ == 0), stop=(dt_ == DT - 1))
            st_["ps"] = ps12

        def part_b():
            ps12 = st_["ps"]
            rstd, nsb = ln_stats_smalls(ps12[:, 0, :], ps12[:, 1, :],
                                        p_l2, "b", AQ, act_smalls=False)
            tmpb = p_l2.tile([P, DT, AQ], BF16, tag="scr8a")
            nc.vector.tensor_tensor(
                tmpb[:], ybf[:, :, qsl],
                rstd[:, None, :].to_broadcast((P, DT, AQ)), OP.mult)
            nc.vector.tensor_tensor(
                fT_bf[:, :, qsl], tmpb[:],
                nsb[:, None, :].to_broadcast((P, DT, AQ)), OP.add)
            with nc.allow_low_precision(reason="f fp8 for fp8 FFN"):
                nc.gpsimd.tensor_tensor(
                    fT8[:, :, qsl], tmpb[:],
                    nsb[:, None, :].to_broadcast((P, DT, AQ)), OP.add)
                if fT8lo is not None:
                    nc.gpsimd.tensor_tensor(fT8lo[:, :, qsl],
                                            fT_bf[:, :, qsl],
                                            fT8[:, :, qsl], OP.subtract)
        return [part_a, part_b]

    def mk_ffn1_task(qc, ft):
        def task():
            qsl = bass.ts(qc, AQ)
            rsl = bass.ts(qc % 2, AQ)     # column range within the pair tile
            w1_s = p_fw1.tile([P, 2, DT, P], F8, tag="w1s")
            nc.sync.dma_start(w1_s[:], w1x_d[:, ft, :, :, :])
            pf = pools_cur["ps"].tile([P, AQ], F32, tag="psg", name="pf")
            mms = [(0, fT8), (1, fT8)]
            if FFN1MM == 3:
                mms.append((0, fT8lo))
            nmm = 0
            tot = 4 * len(mms)
            for hl, rhs in mms:
                for i in range(4):
                    dr(pf[:], w1_s[:, hl, 2 * i:2 * i + 2, :],
                       rhs[:, 2 * i:2 * i + 2, qsl],
                       start=(nmm == 0), stop=(nmm == tot - 1))
                    nmm += 1
            rbf = p_rbf.tile([P, AQ], BF16, tag="rbf")
            nc.vector.tensor_scalar(rbf[:], pf[:], sb_b1[:, ft:ft + 1],
                                    0.0, OP.add, OP.max)
            with nc.allow_low_precision(reason="relu fp8 for fp8 FFN2"):
                nc.vector.tensor_copy(relu8[:, ft, rsl], rbf[:])
                if relu8lo is not None:
                    nc.gpsimd.tensor_tensor(relu8lo[:, ft, rsl], rbf[:],
                                            relu8[:, ft, rsl], OP.subtract)
        return task

    def mk_ffn2_tasks(pr, mt):
        """FFN2 for output tile mt, split into ~1.5us micro-tasks."""
        psl = bass.ts(pr, 512)
        st_ = {}
        # (hl, rhs) matmul units: 32 hi/lo + 16 lo-relu, chunked by 12
        units = ([(0, relu8, j) for j in range(FT // 2)]
                 + [(1, relu8, j) for j in range(FT // 2)])
        if FFN2MM == 3:
            units += [(0, relu8lo, j) for j in range(FT // 2)]

        def c_first():
            w2_s = p_fw.tile([P, 2, FT, P], F8, tag="w2s")
            nc.scalar.dma_start(w2_s[:], w2x_d[:, mt, :, :, :])
            st_["w"] = w2_s
            st_["po"] = pools_cur["ps"].tile([P, 512], F32, tag="psg",
                                             name="po_f2")

        def mk_chunk(lo_i, hi_i, first):
            def chunk():
                if first:
                    c_first()
                w2_s, po = st_["w"], st_["po"]
                for u in range(lo_i, hi_i):
                    hl, rhs, j = units[u]
                    dr(po[:], w2_s[:, hl, 2 * j:2 * j + 2, :],
                       rhs[:, 2 * j:2 * j + 2, :],
                       start=(u == 0), stop=False)
            return chunk

        def c_last():
            w2_s, po = st_["w"], st_["po"]
            nc.tensor.matmul(po[:], sb_g2d[:, mt, :], fT_bf[:, mt, psl],
                             start=False, stop=True, skip_group_check=True)
            ot = p_fo.tile([P, 512], F32, tag="ot")
            nc.vector.tensor_scalar(ot[:], po[:], 1.0 / (WS_FFN1 * WS_FFN2),
                                    sb_b2[:, mt:mt + 1], OP.mult, OP.add)
            nc.scalar.dma_start(OUT_d[:, mt, psl], ot[:])

        n = len(units)
        step = 12
        tasks = []
        for s in range(0, n, step):
            tasks.append(mk_chunk(s, min(s + step, n), s == 0))
        tasks.append(c_last)
        return tasks

    prev = None
    for qc in range(NQC):
        for t in range(DT):
            for i in range(2):
                exh = attn_scores(t, i, qc)
                if prev is not None:
                    attn_v(*prev)
                prev = (t, i, qc, exh)
                drain(4 if qc >= 2 else 3)
    attn_v(*prev)
    prev = None

    # attention psum pools are done; hand their banks to the FFN tail
    close_pool("apsps")
    close_pool("scps")
    p_pst = open_pool("tailps", 4, space="PSUM")
    pools_cur["ps"] = p_pst
    pools_cur["sc"] = p_pst
    drain(bg_len())

    close_pool("fout")
    close_pool("relubf")
    close_pool("relu")
    close_pool("ffnw1")
    close_pool("ffnw")
    close_pool("ln2tmp")
    close_pool("ybp")
    close_pool("xhp")
    close_pool("stage")
    close_pool("expT")
    close_pool("attn")
    close_pool("qkvout")
    close_pool("fTp")
    close_pool("tailps")
    close_pool("psg")
    close_pool("consts")


def _prep_shared(inputs):
    """Host-side weight preprocessing (shared across cores)."""
    f32 = np.float32
    g1 = np.asarray(inputs["g1"], f32)
    beta1 = np.asarray(inputs["beta1"], f32)
    g2 = np.asarray(inputs["g2"], f32)
    beta2 = np.asarray(inputs["beta2"], f32)
    Wq = np.asarray(inputs["Wq"], f32)
    Wk = np.asarray(inputs["Wk"], f32)
    Wv = np.asarray(inputs["Wv"], f32)
    W1 = np.asarray(inputs["W1"], f32)
    W2 = np.asarray(inputs["W2"], f32)

    def fold(Wm, bm):
        Wp = Wm * g1[:, None]
        bp = np.asarray(inputs[bm], f32) + beta1 @ Wm
        return Wp, bp

    Wqp, bqp = fold(Wq, "bq")
    Wkp, bkp = fold(Wk, "bk")
    Wvp, bvp = fold(Wv, "bv")
    W1p = W1 * g2[:, None]
    b1p = np.asarray(inputs["b1"], f32) + beta2 @ W1
    b2p = np.asarray(inputs["b2"], f32) + beta2

    f8 = mybir.dt.np(F8)
    bf = ml_dtypes.bfloat16

    def wtile(Wm, ntile):
        # [K, N] -> [P, ntile, N] with K = ntile*P (partition-major k)
        return np.ascontiguousarray(
            Wm.reshape(ntile, P, Wm.shape[1]).transpose(1, 0, 2))

    def hilo(Wt):
        hi = Wt.astype(f8)
        lo = (Wt - hi.astype(f32)).astype(f8)
        return hi, lo

    wq_t = wtile(WS_QKV * Wqp, DT).astype(f8)
    wk_t = wtile(WS_QKV * Wkp, DT).astype(f8)
    wqk = np.ascontiguousarray(np.stack([wq_t, wk_t], axis=1))
    w1hi, w1lo = hilo(wtile(WS_FFN1 * W1p, DT))
    # slab-contiguous: [P, FT, 2, DT, 128] so one ft slab is one 2KB run
    w1x = np.ascontiguousarray(
        np.stack([w1hi, w1lo], axis=1).reshape(P, 2, DT, FT, P)
        .transpose(0, 3, 1, 2, 4))
    w2hi, w2lo = hilo(wtile(WS_FFN2 * W2, FT))
    # slab-contiguous: [P, DT, 2, FT, 128] so one mt slab is one 8KB run
    w2x = np.ascontiguousarray(
        np.stack([w2hi, w2lo], axis=1).reshape(P, 2, FT, DT, P)
        .transpose(0, 3, 1, 2, 4))

    g2d = np.zeros((P, DT, P), f32)
    for mt in range(DT):
        np.fill_diagonal(g2d[:, mt, :],
                         WS_FFN1 * WS_FFN2 * g2[mt * P:(mt + 1) * P])

    def btile(bv_, ntile):
        return np.ascontiguousarray(bv_.reshape(ntile, P).T).astype(f32)

    E = np.zeros((16, DT, P), f32)
    for t in range(DT):
        for m in range(P):
            E[2 * t + m // HD, t, m] = 1.0

    return {
        "wqk": wqk, "wv": wtile(WS_V * Wvp, DT).astype(f8),
        "w1x": w1x, "w2x": w2x, "g2d": g2d.astype(bf),
        "bq": btile(bqp, DT), "bk": btile(bkp, DT),
        "_bv_fold": btile(bvp, DT),
        "b1": btile(WS_FFN1 * b1p, FT), "b2": btile(b2p, DT),
        "emat": E.astype(bf),
    }


def _per_core_inputs(inputs, shared):
    x = np.asarray(inputs["x"], np.float32)
    f8 = mybir.dt.np(F8)
    bf = ml_dtypes.bfloat16
    maps = []
    for c in range(NCORES):
        b, hf = c // 2, c % 2
        xTn = x[b].T.reshape(DT, P, S).transpose(1, 0, 2)
        if hf == 1:
            xTn = np.concatenate([xTn[:, :, SQ:], xTn[:, :, :SQ]], axis=2)
        xTn = np.ascontiguousarray(xTn)
        m = dict(shared)
        m["xbf"] = xTn.astype(bf)
        m["xf8"] = xTn.astype(f8)
        m["xsq8"] = (xTn * xTn).astype(f8)
        # bv folded into the attention residual (sum of probs == 1)
        m["xh"] = np.ascontiguousarray(
            xTn[:, :, :SQ] + shared["_bv_fold"][:, :, None]).astype(bf)
        maps.append(m)
    return maps


def _get_sharded():
    """Build (once) the nc + jitted shard_map executable."""
    if "sharded" in _CACHE:
        return _CACHE["sharded"]

    import jax
    from jax.sharding import Mesh, PartitionSpec
    from jax.experimental.shard_map import shard_map
    from concourse import bass2jax
    from concourse import mybir as _mybir

    bass2jax.install_neuronx_cc_hook()
    nc = _build_nc()

    partition_name = (nc.partition_id_tensor.name
                      if nc.partition_id_tensor else None)
    in_names, out_names, out_avals, zero_shapes = [], [], [], []
    for alloc in nc.m.functions[0].allocations:
        if not isinstance(alloc, _mybir.MemoryLocationSet):
            continue
        name = alloc.memorylocations[0].name
        if alloc.kind == "ExternalInput":
            if name != partition_name:
                in_names.append(name)
        elif alloc.kind == "ExternalOutput":
            shape = tuple(alloc.tensor_shape)
            dtype = _mybir.dt.np(alloc.dtype)
            out_names.append(name)
            out_avals.append(jax.core.ShapedArray(shape, dtype))
            zero_shapes.append((shape, dtype))
    n_params = len(in_names)
    all_names = in_names + out_names
    if partition_name is not None:
        all_names = all_names + [partition_name]
    donate = tuple(range(n_params, n_params + len(out_names)))

    def _body(*args):
        operands = list(args)
        if partition_name is not None:
            operands.append(bass2jax.partition_id_tensor())
        outs = bass2jax._bass_exec_p.bind(
            *operands,
            out_avals=tuple(out_avals),
            in_names=tuple(all_names),
            out_names=tuple(out_names),
            lowering_input_output_aliases=(),
            sim_require_finite=True,
            sim_require_nnan=True,
            nc=nc,
        )
        return tuple(outs)

    devices = jax.devices()[:NCORES]
    mesh = Mesh(np.asarray(devices), ("core",))
    nin = n_params + len(out_names)
    sharded = jax.jit(
        shard_map(_body, mesh=mesh,
                  in_specs=(PartitionSpec("core"),) * nin,
                  out_specs=(PartitionSpec("core"),) * len(out_names),
                  check_rep=False),
        donate_argnums=donate, keep_unused=True)

    _CACHE["sharded"] = (nc, sharded, in_names, out_names, out_avals,
                         zero_shapes)
    return _CACHE["sharded"]


def _concat_inputs(in_maps):
    _, _, in_names, _, _, zero_shapes = _get_sharded()
    concat_in = [
        np.concatenate([np.asarray(in_maps[c][n]) for c in range(NCORES)],
                       axis=0)
        for n in in_names
    ]
    concat_zeros = [
        np.zeros((NCORES * s[0], *s[1:]), d) for (s, d) in zero_shapes
    ]
    return concat_in, concat_zeros


def _run(in_maps):
    nc, fn, in_names, out_names, out_avals, zero_shapes = _get_sharded()
    concat_in, concat_zeros = _concat_inputs(in_maps)
    outs = fn(*concat_in, *concat_zeros)
    res = []
    for c in range(NCORES):
        res.append({
            name: np.asarray(outs[i]).reshape(NCORES, *out_avals[i].shape)[c]
            for i, name in enumerate(out_names)
        })
    return res


def kernel(**inputs):
    shared = _prep_shared(inputs)
    in_maps = _per_core_inputs(inputs, shared)
    res = _run(in_maps)
    out = np.empty((B, S, D), np.float32)
    for c in range(NCORES):
        b, hf = c // 2, c % 2
        o = res[c]["OUT"]                       # [P, DT, SQ]
        out[b, hf * SQ:(hf + 1) * SQ, :] = o.transpose(2, 1, 0).reshape(SQ, D)
    return out
